# revision 77
# baseline (speedup 1.0000x reference)
"""Trainium2 Bass kernel v4: compacted-key block-causal masked attention.

Module: y = proj(softmax(mask(QK^T/sqrt(D))) V) for B=4, T=2048, C=512, H=8,
with a frame-block-causal mask (frame = t//4) and a per-key validity mask.

Sharding: 8 cores = 4 batches x 2 head-groups (4 heads each); host sums the
two partial output projections per batch and adds the projection bias.

v4 restructures v3 around the TimelineSim cost model:
- K bias is dropped entirely (it only shifts each softmax row uniformly) and
  the Q bias is folded into a host-computed 65th K row (kx = xc^T Wk^T bq s)
  against a constant-ones 65th Q row, so Q/K projection chains need plain
  PSUM->SBUF copies instead of bias adds; scores contract over 65 rows,
  which is free on the PE (cost scales with the free dim only).
- The frame-causality penalty for boundary tiles is a single fp8 DoubleRow
  matmul per (qtile, keytile) pair ([17,2]-contraction one-hot against a
  [17,2,512] query-side pattern) - exact, since all values are 0/1/-192.
- V bias and validity-mask multiplies are gone: padding keys are forced to
  exp()=0 by the penalty rows, and bv's contribution is the constant
  bv @ Wp^T added on the host.
- y^T for the output projection comes from PE transpose-matmuls against an
  identity tile instead of serialized DMA transposes; softmax scaling is one
  DVE multiply with a stride-0 broadcast of the reciprocal denominators.
- Output is written bf16 and summed on the host in f32.
- Emission interleaves projection chains, transposes and output projections
  ("chores", output projections deferred up to 9 qtiles) between att tiles
  so the PE always has independent work while an exp() is in flight; input
  DMAs are merged/ordered against the serialized HWDGE (ones/kx rows ride
  the Pool engine's SWDGE path instead).
"""

import math

import numpy as np

B, T, C = 4, 2048, 512
H, NOBJ, D = 8, 4, 64
NCORES = 8
HPC = 4  # heads per core
NQT = 16  # query tiles of 128
BPT = 8  # score blocks per PSUM att tile ([128, 1024] = 2 banks)
PEN = 192.0  # mask penalty (exactly representable in fp8 e4m3)

_CACHE = {}


def _apply_tile_patch(tile_mod, mybir):
    """walrus in this container rejects >1 semaphore wait per instruction;
    spread the TileContext tail-drain waits over sync NOPs (the rest of the
    module is handled by _split_multi_waits after lowering)."""
    import bass_rust

    if getattr(tile_mod.TileContext, "_drain_patched", False):
        return

    def _drain_and_barrier(self, tick_clock, wait_clock):
        nc = self.nc
        drain_inst = nc.sync.drain()
        wait_clock.add_sem_waits(
            drain_inst.ins, bass_rust.ScopedClock({None: tick_clock.global_clock})
        )
        waits = list(drain_inst.ins.sync_info.on_wait or [])
        if len(waits) > 1:
            drain_inst.ins.sync_info.on_wait = []
            for w in waits:
                nop = nc.sync.nop(nofuse=True)
                nop.ins.sync_info = mybir.SyncInfo(on_wait=[w], on_update=[])
            nc.sync.drain()
        nc.all_engine_barrier()
        assert self.sems is not None
        popped = nc._tile_sem_poison_stack.pop()
        assert popped is self._sem_poison
        nc.clear_and_free_semaphores(list(self.sems.allocated().values()))
        nc.all_engine_barrier()

    tile_mod.TileContext._drain_and_barrier = _drain_and_barrier
    tile_mod.TileContext._drain_patched = True


def _split_multi_waits(nc, mybir):
    """Post-pass: for every instruction carrying more than one semaphore
    wait, hoist the extra waits onto same-engine NOPs inserted immediately
    before it (engines execute serially, so blocking at the NOP is
    equivalent)."""
    nonce = 0
    for fn in nc.m.functions:
        for blk in fn.blocks:
            insts = list(blk.instructions)
            out = []
            changed = False
            for ins in insts:
                si = ins.sync_info
                waits = list(si.on_wait) if si and si.on_wait else []
                if len(waits) > 1:
                    changed = True
                    for w in waits[:-1]:
                        nop = mybir.InstNoOp(
                            name=f"I-waitsplit-{nonce}", ins=[], outs=[]
                        )
                        nonce += 1
                        nop.engine = ins.engine
                        nop.sync_info = mybir.SyncInfo(on_wait=[w], on_update=[])
                        nc.register_instruction(nop, overwrite=True)
                        out.append(nop)
                    ins.sync_info.on_wait = waits[-1:]
                out.append(ins)
            if changed:
                blk.instructions = out


def _schedule(mask):
    """Common (max-over-batch) per-qtile interior/total tile counts."""
    it_c, nt_c = [], []
    kfs = []
    for b in range(B):
        vpos = np.nonzero(mask[b])[0]
        kfs.append(vpos // NOBJ)
    for qt in range(NQT):
        f0, flast = 32 * qt, 32 * qt + 31
        its, nts = [], []
        for b in range(B):
            nf = int(np.searchsorted(kfs[b], f0, "right"))
            na = int(np.searchsorted(kfs[b], flast, "right"))
            its.append(nf // 128)
            nts.append(-(-na // 128))
        it_c.append(min(its))
        nt_c.append(max(max(nts), 1))
    nvt = nt_c[-1]
    pairs = []  # (qt, v) -> aux column index order
    for qt in range(NQT):
        for v in range(it_c[qt], nt_c[qt]):
            pairs.append((qt, v))
    return tuple(it_c), tuple(nt_c), nvt, tuple(pairs)


def _kspans(KW):
    """K projection span widths; a short first span unblocks qt0 early."""
    spans = [(0, 128)]
    off = 128
    while off < KW:
        w = min(512, KW - off)
        spans.append((off, w))
        off += w
    return spans


def _build_program(it_c, nt_c, nvt, pairs):
    import concourse.bass as bass
    import concourse.mybir as mybir
    import concourse.tile as tile

    _apply_tile_patch(tile, mybir)

    f32 = mybir.dt.float32
    bf16 = mybir.dt.bfloat16
    fp8 = mybir.dt.float8e4
    EXP = mybir.ActivationFunctionType.Exp
    DR = mybir.MatmulPerfMode.DoubleRow
    naux = len(pairs)
    pair_idx = {p: i for i, p in enumerate(pairs)}
    KW = nvt * 128  # compacted key width
    NW = 512 + naux * 128  # aux free width per half

    nc = bass.Bass(trn_type="TRN2")

    xt = nc.dram_tensor("xt", [C, T], bf16, kind="ExternalInput")
    xc = nc.dram_tensor("xc", [C, KW], bf16, kind="ExternalInput")
    wb = nc.dram_tensor("wb", [128, 4096], bf16, kind="ExternalInput")
    onesd = nc.dram_tensor("onesd", [1, 4 * T], bf16, kind="ExternalInput")
    kxd = nc.dram_tensor("kxd", [4, KW], bf16, kind="ExternalInput")
    identd = nc.dram_tensor("identd", [128, 128], bf16, kind="ExternalInput")
    auxd = nc.dram_tensor("auxd", [17, 2 * NW], fp8, kind="ExternalInput")
    out = nc.dram_tensor("out", [T, C], bf16, kind="ExternalOutput")

    def mm(o, lhsT, rhs, start, stop, **kw):
        nc.tensor.matmul(o, lhsT, rhs, start=start, stop=stop, **kw)

    kspans = _kspans(KW)

    with nc.allow_low_precision(
        reason="bf16/fp8 matmul inputs; PSUM accumulation stays fp32"
    ), tile.TileContext(nc) as tc:
        with tc.tile_pool(name="const", bufs=1) as cp:
            w_s = cp.tile([128, 4096], bf16)
            xts = cp.tile([128, 4, T], bf16)
            xcs = cp.tile([128, 4, KW], bf16)
            ident = cp.tile([128, 128], bf16)
            aux_s = cp.tile([17, 2, NW], fp8)
            zer_s = cp.tile([1, 128], bf16)
            qtd = cp.tile([65, 4 * T], bf16)
            ktd = cp.tile([65, 4 * KW], bf16)
            v4 = cp.tile([128, nvt, 4, 65], bf16)

            wq_s = w_s[:, 0:1024]
            wk_s = w_s[:, 1024:2048]
            wv_s = w_s[:, 2048:3072]
            wp_s = w_s[:, 3072:4096]

            # ---- input DMAs, ordered by first use (HWDGE serializes) ----
            xtv = xt[:].rearrange("(k p) t -> p k t", p=128)
            xcv = xc[:].rearrange("(k p) t -> p k t", p=128)
            nc.gpsimd.dma_start(ktd[64:65, :], kxd[:])
            nc.gpsimd.dma_start(qtd[64:65, :], onesd[:])
            nc.sync.dma_start(w_s[:, 0:1024], wb[:, 0:1024])
            nc.sync.dma_start(xts[:, :, 0:128], xtv[:, :, 0:128])
            nc.sync.dma_start(xcs[:, :, 0:128], xcv[:, :, 0:128])
            nc.sync.dma_start(w_s[:, 1024:2048], wb[:, 1024:2048])
            nc.sync.dma_start(w_s[:, 2048:3072], wb[:, 2048:3072])
            nc.sync.dma_start(aux_s[:, :, :], auxd[:])
            nc.sync.dma_start(xts[:, :, 128:512], xtv[:, :, 128:512])
            nc.sync.dma_start(xcs[:, :, 128:640], xcv[:, :, 128:640])
            nc.sync.dma_start(xcs[:, :, 640:KW], xcv[:, :, 640:KW])
            nc.sync.dma_start(ident[:], identd[:])
            nc.sync.dma_start(w_s[:, 3072:4096], wb[:, 3072:4096])
            nc.sync.dma_start(xts[:, :, 512:1280], xtv[:, :, 512:1280])
            nc.sync.dma_start(xts[:, :, 1280:2048], xtv[:, :, 1280:2048])
            nc.vector.memset(zer_s[:], 0.0)
            # denominator columns of v4 are constant 1
            nc.vector.memset(v4[:, :, :, 64:65], 1.0)

            with tc.tile_pool(name="attp", bufs=2, space="PSUM") as attp, tc.tile_pool(
                name="yaccp", bufs=2, space="PSUM"
            ) as yaccp, tc.tile_pool(
                name="smp", bufs=2, space="PSUM"
            ) as smp, tc.tile_pool(name="ptp", bufs=3) as ptp, tc.tile_pool(
                name="ysbp", bufs=6
            ) as ysbp, tc.tile_pool(name="rrp", bufs=6) as rrp, tc.tile_pool(
                name="ytsp", bufs=9
            ) as ytsp, tc.tile_pool(name="osp", bufs=9) as osp:
                nonce = [0]

                def emit_q_chain(ps, qc, rc, copy_eng, c0=0, c1=512):
                    w = c1 - c0
                    for kc in range(4):
                        mm(
                            ps[:, 0:w],
                            wq_s[:, kc * 256 + rc * 128 : kc * 256 + rc * 128 + 128],
                            xts[:, kc, qc * 512 + c0 : qc * 512 + c1],
                            kc == 0,
                            kc == 3,
                        )
                    for hh in range(2):
                        h = 2 * rc + hh
                        copy_eng(
                            qtd[0:64, h * T + qc * 512 + c0 : h * T + qc * 512 + c1],
                            ps[hh * 64 : (hh + 1) * 64, 0:w],
                        )

                def emit_k_chain(ps, si, rc, copy_eng):
                    off, w = kspans[si]
                    for kc in range(4):
                        mm(
                            ps[:, 0:w],
                            wk_s[:, kc * 256 + rc * 128 : kc * 256 + rc * 128 + 128],
                            xcs[:, kc, off : off + w],
                            kc == 0,
                            kc == 3,
                        )
                    for hh in range(2):
                        h = 2 * rc + hh
                        copy_eng(
                            ktd[0:64, h * KW + off : h * KW + off + w],
                            ps[hh * 64 : (hh + 1) * 64, 0:w],
                        )

                def emit_v_chain(ps, kb, copy_eng):
                    for kc in range(4):
                        mm(
                            ps[:, 0:256],
                            xcs[:, kc, kb * 128 : kb * 128 + 128],
                            wv_s[:, kc * 256 : (kc + 1) * 256],
                            kc == 0,
                            kc == 3,
                        )
                    copy_eng(v4[:, kb, :, 0:64], ps[:, 0:256])

                def emit_unit(u, ps=None):
                    if ps is None:
                        ps = smp.tile([128, 512], f32, tag="sm", name=f"u{nonce[0]}")
                        nonce[0] += 1
                    kind = u[0]
                    if kind == "q":
                        emit_q_chain(ps, u[1], u[2], nc.vector.tensor_copy)
                    elif kind == "q0r":
                        emit_q_chain(ps, 0, u[1], nc.vector.tensor_copy, 128, 512)
                    elif kind == "k":
                        emit_k_chain(ps, u[1], u[2], nc.vector.tensor_copy)
                    else:
                        emit_v_chain(ps, u[1], nc.vector.tensor_copy)

                def qt_tiles(qt):
                    """Slot items per att tile: interior head-blocks first,
                    then 4-aligned boundary groups."""
                    items = [("blk", h, kb) for h in range(HPC) for kb in range(it_c[qt])]
                    for v in range(it_c[qt], nt_c[qt]):
                        items.append(("grp", v))
                    tiles = []
                    cur, used = [], 0
                    for itm in items:
                        sz = 4 if itm[0] == "grp" else 1
                        if used + sz > BPT:
                            tiles.append((cur, used))
                            cur, used = [], 0
                        cur.append((used, itm))
                        used += sz
                    if cur:
                        tiles.append((cur, used))
                    return tiles

                def emit_scores(qt, at, take):
                    for slot, itm in take:
                        if itm[0] == "blk":
                            _, h, kb = itm
                            mm(
                                at[:, slot * 128 : (slot + 1) * 128],
                                ktd[0:65, h * KW + kb * 128 : h * KW + (kb + 1) * 128],
                                qtd[0:65, h * T + qt * 128 : h * T + (qt + 1) * 128],
                                True,
                                True,
                            )
                        else:
                            v = itm[1]
                            for hh in range(HPC):
                                mm(
                                    at[:, (slot + hh) * 128 : (slot + hh + 1) * 128],
                                    ktd[0:65, hh * KW + v * 128 : hh * KW + (v + 1) * 128],
                                    qtd[0:65, hh * T + qt * 128 : hh * T + (qt + 1) * 128],
                                    hh == 0,
                                    False,
                                    skip_group_check=True,
                                )
                            p = pair_idx[(qt, v)]
                            mm(
                                at[:, slot * 128 : (slot + 4) * 128],
                                aux_s[:, :, 512 + p * 128 : 512 + (p + 1) * 128],
                                aux_s[:, :, 0:512],
                                False,
                                True,
                                perf_mode=DR,
                                skip_group_check=True,
                            )

                def emit_pv_tile(qt, yacc, pt, take):
                    nt = nt_c[qt]
                    for slot, itm in take:
                        if itm[0] == "blk":
                            blocks = [(slot, itm[1], itm[2])]
                        else:
                            blocks = [(slot + hh, hh, itm[1]) for hh in range(HPC)]
                        for sl, h, kb in blocks:
                            mm(
                                yacc[:, h * 65 : h * 65 + 65],
                                pt[:, sl * 128 : (sl + 1) * 128],
                                v4[:, kb, h, :],
                                False,
                                kb == nt - 1,
                                skip_group_check=True,
                            )

                ysb_t = {}
                yts_t = {}

                def emit_fin(qt, yacc):
                    """Recip the denominators, scale all heads into bf16 with
                    one broadcast multiply."""
                    rr = rrp.tile([128, 4], f32, tag="rr", name=f"rr{qt}")
                    nc.vector.reciprocal(rr[:], yacc[:, 64:260:65])
                    ysb = ysbp.tile([128, 256], bf16, tag="ysb", name=f"ysb{qt}")
                    yv = yacc.rearrange("p (h d) -> p h d", h=4)[:, :, 0:64]
                    nc.vector.tensor_mul(
                        ysb[:].rearrange("p (h d) -> p h d", h=4),
                        yv,
                        rr[:].unsqueeze(2).broadcast_to([128, 4, 64]),
                    )
                    ysb_t[qt] = ysb

                def emit_trans(qt):
                    ysb = ysb_t.pop(qt)
                    ytp = yaccp.tile([128, 256], bf16, tag="yacc", name=f"ytp{qt}")
                    nc.tensor.transpose(ytp[:, 0:128], ysb[:, 0:128], ident[:])
                    nc.tensor.transpose(ytp[:, 128:256], ysb[:, 128:256], ident[:])
                    yts = ytsp.tile([128, 256], bf16, tag="yts", name=f"yts{qt}")
                    nc.vector.tensor_copy(yts[:], ytp[:])
                    yts_t[qt] = yts

                def emit_proj(qt):
                    yts = yts_t.pop(qt)
                    po = smp.tile([128, 512], f32, tag="sm", name=f"po{qt}")
                    mm(po[:], yts[:, 0:128], wp_s[:, 0:512], True, False)
                    mm(po[:], yts[:, 128:256], wp_s[:, 512:1024], False, True)
                    os_t = osp.tile([128, 512], bf16, tag="os", name=f"os{qt}")
                    nc.vector.tensor_copy(os_t[:], po[:])
                    nc.sync.dma_start(out[qt * 128 : (qt + 1) * 128, :], os_t[:])

                # chores: (deadline qt, pe_ns, thunk) consumed between att
                # tiles so the PE always has independent work while exp()
                # runs; pop ~a tile-period's worth of PE work each time.
                chores = []

                def pop_chore(qt):
                    if chores and chores[0][0] <= qt + 1:
                        chores.pop(0)[2]()

                def flush_chores(qt):
                    while chores and chores[0][0] <= qt:
                        chores.pop(0)[2]()

                def emit_attention(qt):
                    tiles = qt_tiles(qt)
                    yacc = yaccp.tile([128, 260], f32, tag="yacc", name=f"yacc{qt}")
                    # zero-init the accumulator bank with a 1-row matmul
                    mm(yacc[:, 0:260], zer_s[:], zbias_s[:, 0:260], True, False,
                       skip_group_check=True)
                    ptiles = []
                    for t, (tb, used) in enumerate(tiles):
                        at = attp.tile([128, BPT * 128], f32, tag="att", name=f"at{qt}_{t}")
                        emit_scores(qt, at, tb)
                        pt = ptp.tile([128, BPT * 128], bf16, tag="p", name=f"p{qt}_{t}")
                        nc.scalar.activation(pt[:, 0 : used * 128], at[:, 0 : used * 128], EXP)
                        ptiles.append((pt, tb))
                        pop_chore(qt)
                        if t >= 1:
                            emit_pv_tile(qt, yacc, *ptiles[t - 1])
                    emit_pv_tile(qt, yacc, *ptiles[-1])
                    emit_fin(qt, yacc)
                    pop_chore(qt)

                zbias_s = cp.tile([1, 260], bf16)
                nc.vector.memset(zbias_s[:], 0.0)

                # ---- prelude: chains needed by the first query tiles ----
                # park early chains in the idle att-psum banks to run them
                # in parallel; the rest flow through the sm pool.
                pre1 = attp.tile([128, BPT * 128], f32, tag="att", name="pre1")
                emit_q_chain(pre1[:, 0:512], 0, 0, nc.vector.tensor_copy, 0, 128)
                pre2 = attp.tile([128, BPT * 128], f32, tag="att", name="pre2")
                emit_q_chain(pre2[:, 0:512], 0, 1, nc.vector.tensor_copy, 0, 128)
                emit_k_chain(pre1[:, 512:1024], 0, 0, nc.scalar.copy)
                emit_k_chain(pre2[:, 512:1024], 0, 1, nc.scalar.copy)
                emit_unit(("v", 0))

                # remaining chains with deadlines (emit before attention(d))
                chores.append((1, 860, lambda: emit_unit(("q0r", 0))))
                chores.append((1, 860, lambda: emit_unit(("q0r", 1))))
                for si in range(1, len(kspans)):
                    lo = kspans[si][0] // 128
                    d = next((q for q in range(NQT) if nt_c[q] > lo), NQT - 1)
                    chores.append((d, 860, lambda s=si: emit_unit(("k", s, 0))))
                    chores.append((d, 860, lambda s=si: emit_unit(("k", s, 1))))
                for kb in range(1, nvt):
                    d = next((q for q in range(NQT) if nt_c[q] > kb), NQT - 1)
                    chores.append((d, 860, lambda k=kb: emit_unit(("v", k))))
                for qc in range(1, 4):
                    chores.append((4 * qc, 860, lambda q=qc: emit_unit(("q", q, 0))))
                    chores.append((4 * qc, 860, lambda q=qc: emit_unit(("q", q, 1))))
                chores.sort(key=lambda x: x[0])

                for qt in range(NQT):
                    flush_chores(qt)
                    emit_attention(qt)
                    if qt >= 1:
                        # y-transpose and projection of the previous qtile
                        # ride the chore queue to fill later PE gaps
                        chores.append((qt + (1 if qt < 12 else 2), 150, lambda q=qt - 1: emit_trans(q)))
                        chores.append((qt + 9, 430, lambda q=qt - 1: emit_proj(q)))
                        chores.sort(key=lambda x: x[0])
                flush_chores(10**9)
                emit_trans(NQT - 1)
                emit_proj(NQT - 1)
                assert not chores and not ysb_t and not yts_t, (
                    "unemitted work left behind",
                    len(chores), list(ysb_t), list(yts_t),
                )
    _split_multi_waits(nc, mybir)
    return nc


def _host_inputs(x, mask, Wq, bq, Wk, bk, Wv, bv, Wp, bp, sched):
    """Build the per-core input maps."""
    import ml_dtypes

    bf = ml_dtypes.bfloat16
    f8 = ml_dtypes.float8_e4m3
    it_c, nt_c, nvt, pairs = sched
    naux = len(pairs)
    KW = nvt * 128
    NW = 512 + naux * 128
    scale = 1.0 / math.sqrt(D)

    # query-side penalty pattern aq4 [34, 512]: -PEN where row > lf(q),
    # replicated over the 4 head slots; row 32 = always-mask, row 33 = pad.
    lf = np.arange(128) // NOBJ
    r = np.arange(34)[:, None]
    aq1 = np.where((r > lf[None, :]) & (r < 33), np.float32(-PEN), np.float32(0.0))
    aq4 = np.tile(aq1, (1, 4))  # [34, 512]

    ident = np.eye(128, dtype=np.float32)
    ones_host = np.ones((1, 4 * T), np.float32)

    in_maps = []
    for c in range(NCORES):
        b, g = divmod(c, 2)
        ch = slice(g * 256, (g + 1) * 256)
        vpos = np.nonzero(mask[b])[0]
        nv = len(vpos)
        kf = vpos // NOBJ
        # compacted x for K/V, zero-padded
        xc_h = np.zeros((C, KW), np.float32)
        xc_h[:, :nv] = x[b].T[:, vpos]
        # aux: [34, NW] -> DoubleRow layout [17, 2, NW]
        aux_h = np.zeros((34, NW), np.float32)
        aux_h[:, 0:512] = aq4
        for p, (qt, v) in enumerate(pairs):
            f0 = 32 * qt
            col0 = 512 + p * 128
            for kk in range(128):
                j = v * 128 + kk
                if j >= nv:
                    aux_h[32, col0 + kk] = 1.0  # padding: always masked
                    continue
                i = int(kf[j]) - f0
                if i <= 0:
                    continue  # interior key: no penalty
                aux_h[min(i, 32), col0 + kk] = 1.0
        aux_dr = aux_h.reshape(2, 17, NW).transpose(1, 0, 2).reshape(17, 2 * NW)

        wq_h = np.ascontiguousarray((Wq[ch, :] * scale).T)  # [C, 256]
        wk_h = np.ascontiguousarray(Wk[ch, :].T)
        wv_h = np.ascontiguousarray(Wv[ch, :].T)
        wp_h = np.ascontiguousarray(Wp[:, ch].T)  # [256, 512]
        # pack [128, 4096]: per-kc 256-wide blocks for wq/wk/wv, rc-major wp
        wb_h = np.zeros((128, 4096), np.float32)
        for kc in range(4):
            wb_h[:, kc * 256 : (kc + 1) * 256] = wq_h[kc * 128 : (kc + 1) * 128, :]
            wb_h[:, 1024 + kc * 256 : 1024 + (kc + 1) * 256] = wk_h[
                kc * 128 : (kc + 1) * 128, :
            ]
            wb_h[:, 2048 + kc * 256 : 2048 + (kc + 1) * 256] = wv_h[
                kc * 128 : (kc + 1) * 128, :
            ]
        for rc in range(2):
            wb_h[:, 3072 + rc * 512 : 3072 + (rc + 1) * 512] = wp_h[
                rc * 128 : (rc + 1) * 128, :
            ]

        # kx[h] = xc^T (Wk_h^T (s bq_h)) : folds the Q bias into a 65th K row
        bqs = bq[ch] * scale
        kx_h = np.zeros((4, KW), np.float32)
        for h in range(HPC):
            wkh = Wk[ch, :][h * 64 : (h + 1) * 64, :]  # [64, C]
            w_vec = wkh.T @ bqs[h * 64 : (h + 1) * 64]  # [C]
            kx_h[h] = xc_h.T @ w_vec

        in_maps.append(
            {
                "xt": np.ascontiguousarray(x[b].T).astype(bf),
                "xc": xc_h.astype(bf),
                "wb": wb_h.astype(bf),
                "onesd": ones_host.astype(bf),
                "kxd": kx_h.astype(bf),
                "identd": ident.astype(bf),
                "auxd": aux_dr.astype(f8),
            }
        )
    return in_maps


def kernel(x, mask, Wq, bq, Wk, bk, Wv, bv, Wp, bp):
    from concourse.bass_utils import run_bass_kernel_spmd

    mask = np.asarray(mask)
    sched = _schedule(mask)
    key = ("nc", sched)
    if key not in _CACHE:
        _CACHE[key] = _build_program(*sched)
        _CACHE["nc"] = _CACHE[key]  # for test harness introspection
    nc = _CACHE[key]

    x, Wq, bq, Wk, bk = map(np.asarray, (x, Wq, bq, Wk, bk))
    Wv, bv, Wp, bp = map(np.asarray, (Wv, bv, Wp, bp))
    in_maps = _host_inputs(x, mask, Wq, bq, Wk, bk, Wv, bv, Wp, bp, sched)
    res = run_bass_kernel_spmd(nc, in_maps, core_ids=list(range(NCORES)))
    outs = [res.results[c]["out"] for c in range(NCORES)]
    const = bp + bv @ Wp.T  # bias contributions folded out of the device pass
    y = np.empty((B, T, C), np.float32)
    for b in range(B):
        y[b] = (
            outs[2 * b].astype(np.float32)
            + outs[2 * b + 1].astype(np.float32)
            + const[None, :]
        )
    return y


# revision 87
# speedup vs baseline: 1.0005x; 1.0005x over previous
"""Trainium2 Bass kernel v4: compacted-key block-causal masked attention.

Module: y = proj(softmax(mask(QK^T/sqrt(D))) V) for B=4, T=2048, C=512, H=8,
with a frame-block-causal mask (frame = t//4) and a per-key validity mask.

Sharding: 8 cores = 4 batches x 2 head-groups (4 heads each); host sums the
two partial output projections per batch and adds the projection bias.

v4 restructures v3 around the TimelineSim cost model:
- K bias is dropped entirely (it only shifts each softmax row uniformly) and
  the Q bias is folded into a host-computed 65th K row (kx = xc^T Wk^T bq s)
  against a constant-ones 65th Q row, so Q/K projection chains need plain
  PSUM->SBUF copies instead of bias adds; scores contract over 65 rows,
  which is free on the PE (cost scales with the free dim only).
- The frame-causality penalty for boundary tiles is a single fp8 DoubleRow
  matmul per (qtile, keytile) pair ([17,2]-contraction one-hot against a
  [17,2,512] query-side pattern) - exact, since all values are 0/1/-192.
- V bias and validity-mask multiplies are gone: padding keys are forced to
  exp()=0 by the penalty rows, and bv's contribution is the constant
  bv @ Wp^T added on the host.
- y^T for the output projection comes from PE transpose-matmuls against an
  identity tile instead of serialized DMA transposes; softmax scaling is one
  DVE multiply with a stride-0 broadcast of the reciprocal denominators.
- Output is written bf16 and summed on the host in f32.
- Emission interleaves projection chains, transposes and output projections
  ("chores", output projections deferred up to 9 qtiles) between att tiles
  so the PE always has independent work while an exp() is in flight; input
  DMAs are merged/ordered against the serialized HWDGE (ones/kx rows ride
  the Pool engine's SWDGE path instead).
"""

import math

import numpy as np

B, T, C = 4, 2048, 512
H, NOBJ, D = 8, 4, 64
NCORES = 8
HPC = 4  # heads per core
NQT = 16  # query tiles of 128
BPT = 8  # score blocks per PSUM att tile ([128, 1024] = 2 banks)
PEN = 192.0  # mask penalty (exactly representable in fp8 e4m3)

_CACHE = {}


def _apply_tile_patch(tile_mod, mybir):
    """walrus in this container rejects >1 semaphore wait per instruction;
    spread the TileContext tail-drain waits over sync NOPs (the rest of the
    module is handled by _split_multi_waits after lowering)."""
    import bass_rust

    if getattr(tile_mod.TileContext, "_drain_patched", False):
        return

    def _drain_and_barrier(self, tick_clock, wait_clock):
        nc = self.nc
        drain_inst = nc.sync.drain()
        wait_clock.add_sem_waits(
            drain_inst.ins, bass_rust.ScopedClock({None: tick_clock.global_clock})
        )
        waits = list(drain_inst.ins.sync_info.on_wait or [])
        if len(waits) > 1:
            drain_inst.ins.sync_info.on_wait = []
            for w in waits:
                nop = nc.sync.nop(nofuse=True)
                nop.ins.sync_info = mybir.SyncInfo(on_wait=[w], on_update=[])
            nc.sync.drain()
        nc.all_engine_barrier()
        assert self.sems is not None
        popped = nc._tile_sem_poison_stack.pop()
        assert popped is self._sem_poison
        nc.clear_and_free_semaphores(list(self.sems.allocated().values()))
        nc.all_engine_barrier()

    tile_mod.TileContext._drain_and_barrier = _drain_and_barrier
    tile_mod.TileContext._drain_patched = True


def _split_multi_waits(nc, mybir):
    """Post-pass: for every instruction carrying more than one semaphore
    wait, hoist the extra waits onto same-engine NOPs inserted immediately
    before it (engines execute serially, so blocking at the NOP is
    equivalent)."""
    nonce = 0
    for fn in nc.m.functions:
        for blk in fn.blocks:
            insts = list(blk.instructions)
            out = []
            changed = False
            for ins in insts:
                si = ins.sync_info
                waits = list(si.on_wait) if si and si.on_wait else []
                if len(waits) > 1:
                    changed = True
                    for w in waits[:-1]:
                        nop = mybir.InstNoOp(
                            name=f"I-waitsplit-{nonce}", ins=[], outs=[]
                        )
                        nonce += 1
                        nop.engine = ins.engine
                        nop.sync_info = mybir.SyncInfo(on_wait=[w], on_update=[])
                        nc.register_instruction(nop, overwrite=True)
                        out.append(nop)
                    ins.sync_info.on_wait = waits[-1:]
                out.append(ins)
            if changed:
                blk.instructions = out


def _schedule(mask):
    """Common (max-over-batch) per-qtile interior/total tile counts."""
    it_c, nt_c = [], []
    kfs = []
    for b in range(B):
        vpos = np.nonzero(mask[b])[0]
        kfs.append(vpos // NOBJ)
    for qt in range(NQT):
        f0, flast = 32 * qt, 32 * qt + 31
        its, nts = [], []
        for b in range(B):
            nf = int(np.searchsorted(kfs[b], f0, "right"))
            na = int(np.searchsorted(kfs[b], flast, "right"))
            its.append(nf // 128)
            nts.append(-(-na // 128))
        it_c.append(min(its))
        nt_c.append(max(max(nts), 1))
    nvt = nt_c[-1]
    pairs = []  # (qt, v) -> aux column index order
    for qt in range(NQT):
        for v in range(it_c[qt], nt_c[qt]):
            pairs.append((qt, v))
    return tuple(it_c), tuple(nt_c), nvt, tuple(pairs)


def _kspans(KW):
    """K projection span widths; a short first span unblocks qt0 early."""
    spans = [(0, 128)]
    off = 128
    while off < KW:
        w = min(512, KW - off)
        spans.append((off, w))
        off += w
    return spans


def _build_program(it_c, nt_c, nvt, pairs):
    import concourse.bass as bass
    import concourse.mybir as mybir
    import concourse.tile as tile

    _apply_tile_patch(tile, mybir)

    f32 = mybir.dt.float32
    bf16 = mybir.dt.bfloat16
    fp8 = mybir.dt.float8e4
    EXP = mybir.ActivationFunctionType.Exp
    DR = mybir.MatmulPerfMode.DoubleRow
    naux = len(pairs)
    pair_idx = {p: i for i, p in enumerate(pairs)}
    KW = nvt * 128  # compacted key width
    NW = 512 + naux * 128  # aux free width per half

    nc = bass.Bass(trn_type="TRN2")

    xt = nc.dram_tensor("xt", [C, T], bf16, kind="ExternalInput")
    xc = nc.dram_tensor("xc", [C, KW], bf16, kind="ExternalInput")
    wb = nc.dram_tensor("wb", [128, 4096], bf16, kind="ExternalInput")
    onesd = nc.dram_tensor("onesd", [1, 4 * T], bf16, kind="ExternalInput")
    kxd = nc.dram_tensor("kxd", [4, KW], bf16, kind="ExternalInput")
    identd = nc.dram_tensor("identd", [128, 128], bf16, kind="ExternalInput")
    auxd = nc.dram_tensor("auxd", [17, 2 * NW], fp8, kind="ExternalInput")
    out = nc.dram_tensor("out", [T, C], bf16, kind="ExternalOutput")

    def mm(o, lhsT, rhs, start, stop, **kw):
        nc.tensor.matmul(o, lhsT, rhs, start=start, stop=stop, **kw)

    kspans = _kspans(KW)

    with nc.allow_low_precision(
        reason="bf16/fp8 matmul inputs; PSUM accumulation stays fp32"
    ), tile.TileContext(nc) as tc:
        with tc.tile_pool(name="const", bufs=1) as cp:
            w_s = cp.tile([128, 4096], bf16)
            xts = cp.tile([128, 4, T], bf16)
            xcs = cp.tile([128, 4, KW], bf16)
            ident = cp.tile([128, 128], bf16)
            aux_s = cp.tile([17, 2, NW], fp8)
            zer_s = cp.tile([1, 128], bf16)
            qtd = cp.tile([65, 4 * T], bf16)
            ktd = cp.tile([65, 4 * KW], bf16)
            v4 = cp.tile([128, nvt, 4, 65], bf16)

            wq_s = w_s[:, 0:1024]
            wk_s = w_s[:, 1024:2048]
            wv_s = w_s[:, 2048:3072]
            wp_s = w_s[:, 3072:4096]

            # ---- input DMAs, ordered by first use (HWDGE serializes) ----
            xtv = xt[:].rearrange("(k p) t -> p k t", p=128)
            xcv = xc[:].rearrange("(k p) t -> p k t", p=128)
            nc.gpsimd.dma_start(ktd[64:65, :], kxd[:])
            nc.gpsimd.dma_start(qtd[64:65, :], onesd[:])
            nc.sync.dma_start(w_s[:, 0:1024], wb[:, 0:1024])
            nc.sync.dma_start(xts[:, :, 0:128], xtv[:, :, 0:128])
            nc.sync.dma_start(xcs[:, :, 0:128], xcv[:, :, 0:128])
            nc.sync.dma_start(w_s[:, 1024:2048], wb[:, 1024:2048])
            nc.sync.dma_start(w_s[:, 2048:3072], wb[:, 2048:3072])
            nc.sync.dma_start(aux_s[:, :, :], auxd[:])
            nc.sync.dma_start(xts[:, :, 128:512], xtv[:, :, 128:512])
            nc.sync.dma_start(xcs[:, :, 128:640], xcv[:, :, 128:640])
            nc.sync.dma_start(xcs[:, :, 640:KW], xcv[:, :, 640:KW])
            nc.sync.dma_start(ident[:], identd[:])
            nc.sync.dma_start(w_s[:, 3072:4096], wb[:, 3072:4096])
            nc.sync.dma_start(xts[:, :, 512:1280], xtv[:, :, 512:1280])
            nc.sync.dma_start(xts[:, :, 1280:2048], xtv[:, :, 1280:2048])
            nc.vector.memset(zer_s[:], 0.0)
            # denominator columns of v4 are constant 1
            nc.vector.memset(v4[:, :, :, 64:65], 1.0)

            with tc.tile_pool(name="attp", bufs=2, space="PSUM") as attp, tc.tile_pool(
                name="yaccp", bufs=2, space="PSUM"
            ) as yaccp, tc.tile_pool(
                name="smp", bufs=2, space="PSUM"
            ) as smp, tc.tile_pool(name="ptp", bufs=3) as ptp, tc.tile_pool(
                name="ysbp", bufs=6
            ) as ysbp, tc.tile_pool(name="rrp", bufs=6) as rrp, tc.tile_pool(
                name="ytsp", bufs=9
            ) as ytsp, tc.tile_pool(name="osp", bufs=9) as osp:
                nonce = [0]

                def emit_q_chain(ps, qc, rc, copy_eng, c0=0, c1=512):
                    w = c1 - c0
                    for kc in range(4):
                        mm(
                            ps[:, 0:w],
                            wq_s[:, kc * 256 + rc * 128 : kc * 256 + rc * 128 + 128],
                            xts[:, kc, qc * 512 + c0 : qc * 512 + c1],
                            kc == 0,
                            kc == 3,
                        )
                    for hh in range(2):
                        h = 2 * rc + hh
                        copy_eng(
                            qtd[0:64, h * T + qc * 512 + c0 : h * T + qc * 512 + c1],
                            ps[hh * 64 : (hh + 1) * 64, 0:w],
                        )

                def emit_k_chain(ps, si, rc, copy_eng):
                    off, w = kspans[si]
                    for kc in range(4):
                        mm(
                            ps[:, 0:w],
                            wk_s[:, kc * 256 + rc * 128 : kc * 256 + rc * 128 + 128],
                            xcs[:, kc, off : off + w],
                            kc == 0,
                            kc == 3,
                        )
                    for hh in range(2):
                        h = 2 * rc + hh
                        copy_eng(
                            ktd[0:64, h * KW + off : h * KW + off + w],
                            ps[hh * 64 : (hh + 1) * 64, 0:w],
                        )

                def emit_v_chain(ps, kb, copy_eng):
                    for kc in range(4):
                        mm(
                            ps[:, 0:256],
                            xcs[:, kc, kb * 128 : kb * 128 + 128],
                            wv_s[:, kc * 256 : (kc + 1) * 256],
                            kc == 0,
                            kc == 3,
                        )
                    copy_eng(v4[:, kb, :, 0:64], ps[:, 0:256])

                def emit_unit(u, ps=None):
                    if ps is None:
                        ps = smp.tile([128, 512], f32, tag="sm", name=f"u{nonce[0]}")
                        nonce[0] += 1
                    kind = u[0]
                    if kind == "q":
                        emit_q_chain(ps, u[1], u[2], nc.vector.tensor_copy)
                    elif kind == "q0r":
                        emit_q_chain(ps, 0, u[1], nc.vector.tensor_copy, 128, 512)
                    elif kind == "k":
                        emit_k_chain(ps, u[1], u[2], nc.vector.tensor_copy)
                    else:
                        emit_v_chain(ps, u[1], nc.vector.tensor_copy)

                def qt_tiles(qt):
                    """Slot items per att tile: interior head-blocks first,
                    then 4-aligned boundary groups."""
                    items = [("blk", h, kb) for h in range(HPC) for kb in range(it_c[qt])]
                    for v in range(it_c[qt], nt_c[qt]):
                        items.append(("grp", v))
                    tiles = []
                    cur, used = [], 0
                    for itm in items:
                        sz = 4 if itm[0] == "grp" else 1
                        if used + sz > BPT:
                            tiles.append((cur, used))
                            cur, used = [], 0
                        cur.append((used, itm))
                        used += sz
                    if cur:
                        tiles.append((cur, used))
                    return tiles

                def emit_scores(qt, at, take):
                    for slot, itm in take:
                        if itm[0] == "blk":
                            _, h, kb = itm
                            mm(
                                at[:, slot * 128 : (slot + 1) * 128],
                                ktd[0:65, h * KW + kb * 128 : h * KW + (kb + 1) * 128],
                                qtd[0:65, h * T + qt * 128 : h * T + (qt + 1) * 128],
                                True,
                                True,
                            )
                        else:
                            v = itm[1]
                            for hh in range(HPC):
                                mm(
                                    at[:, (slot + hh) * 128 : (slot + hh + 1) * 128],
                                    ktd[0:65, hh * KW + v * 128 : hh * KW + (v + 1) * 128],
                                    qtd[0:65, hh * T + qt * 128 : hh * T + (qt + 1) * 128],
                                    hh == 0,
                                    False,
                                    skip_group_check=True,
                                )
                            p = pair_idx[(qt, v)]
                            mm(
                                at[:, slot * 128 : (slot + 4) * 128],
                                aux_s[:, :, 512 + p * 128 : 512 + (p + 1) * 128],
                                aux_s[:, :, 0:512],
                                False,
                                True,
                                perf_mode=DR,
                                skip_group_check=True,
                            )

                def emit_pv_tile(qt, yacc, pt, take):
                    nt = nt_c[qt]
                    for slot, itm in take:
                        if itm[0] == "blk":
                            blocks = [(slot, itm[1], itm[2])]
                        else:
                            blocks = [(slot + hh, hh, itm[1]) for hh in range(HPC)]
                        for sl, h, kb in blocks:
                            mm(
                                yacc[:, h * 65 : h * 65 + 65],
                                pt[:, sl * 128 : (sl + 1) * 128],
                                v4[:, kb, h, :],
                                False,
                                kb == nt - 1,
                                skip_group_check=True,
                            )

                ysb_t = {}
                yts_t = {}

                def emit_fin(qt, yacc):
                    """Recip the denominators, scale all heads into bf16 with
                    one broadcast multiply."""
                    rr = rrp.tile([128, 4], f32, tag="rr", name=f"rr{qt}")
                    nc.vector.reciprocal(rr[:], yacc[:, 64:260:65])
                    ysb = ysbp.tile([128, 256], bf16, tag="ysb", name=f"ysb{qt}")
                    yv = yacc.rearrange("p (h d) -> p h d", h=4)[:, :, 0:64]
                    nc.vector.tensor_mul(
                        ysb[:].rearrange("p (h d) -> p h d", h=4),
                        yv,
                        rr[:].unsqueeze(2).broadcast_to([128, 4, 64]),
                    )
                    ysb_t[qt] = ysb

                def emit_trans(qt):
                    ysb = ysb_t.pop(qt)
                    ytp = yaccp.tile([128, 256], bf16, tag="yacc", name=f"ytp{qt}")
                    nc.tensor.transpose(ytp[:, 0:128], ysb[:, 0:128], ident[:])
                    nc.tensor.transpose(ytp[:, 128:256], ysb[:, 128:256], ident[:])
                    yts = ytsp.tile([128, 256], bf16, tag="yts", name=f"yts{qt}")
                    nc.vector.tensor_copy(yts[:], ytp[:])
                    yts_t[qt] = yts

                def emit_proj(qt):
                    yts = yts_t.pop(qt)
                    po = smp.tile([128, 512], f32, tag="sm", name=f"po{qt}")
                    mm(po[:], yts[:, 0:128], wp_s[:, 0:512], True, False)
                    mm(po[:], yts[:, 128:256], wp_s[:, 512:1024], False, True)
                    os_t = osp.tile([128, 512], bf16, tag="os", name=f"os{qt}")
                    nc.vector.tensor_copy(os_t[:], po[:])
                    nc.sync.dma_start(out[qt * 128 : (qt + 1) * 128, :], os_t[:])

                # chores: (deadline qt, pe_ns, thunk) consumed between att
                # tiles so the PE always has independent work while exp()
                # runs; pop ~a tile-period's worth of PE work each time.
                chores = []

                def pop_chore(qt):
                    if chores and chores[0][0] <= qt + 1:
                        chores.pop(0)[2]()

                def flush_chores(qt):
                    while chores and chores[0][0] <= qt:
                        chores.pop(0)[2]()

                def emit_attention(qt):
                    tiles = qt_tiles(qt)
                    yacc = yaccp.tile([128, 260], f32, tag="yacc", name=f"yacc{qt}")
                    ptiles = []
                    for t, (tb, used) in enumerate(tiles):
                        at = attp.tile([128, BPT * 128], f32, tag="att", name=f"at{qt}_{t}")
                        emit_scores(qt, at, tb)
                        if t == 0:
                            # zero-init the accumulator bank with a 1-row
                            # matmul, deferred past the first scores so it
                            # never head-blocks the PE at qtile start
                            mm(yacc[:, 0:260], zer_s[:], zbias_s[:, 0:260],
                               True, False, skip_group_check=True)
                        pt = ptp.tile([128, BPT * 128], bf16, tag="p", name=f"p{qt}_{t}")
                        nc.scalar.activation(pt[:, 0 : used * 128], at[:, 0 : used * 128], EXP)
                        ptiles.append((pt, tb))
                        pop_chore(qt)
                        if t >= 1:
                            emit_pv_tile(qt, yacc, *ptiles[t - 1])
                    emit_pv_tile(qt, yacc, *ptiles[-1])
                    emit_fin(qt, yacc)
                    pop_chore(qt)

                zbias_s = cp.tile([1, 260], bf16)
                nc.vector.memset(zbias_s[:], 0.0)

                # ---- prelude: chains needed by the first query tiles ----
                # park early chains in the idle att-psum banks to run them
                # in parallel; the rest flow through the sm pool.
                pre1 = attp.tile([128, BPT * 128], f32, tag="att", name="pre1")
                emit_q_chain(pre1[:, 0:512], 0, 0, nc.vector.tensor_copy, 0, 128)
                pre2 = attp.tile([128, BPT * 128], f32, tag="att", name="pre2")
                emit_q_chain(pre2[:, 0:512], 0, 1, nc.vector.tensor_copy, 0, 128)
                emit_k_chain(pre1[:, 512:1024], 0, 0, nc.scalar.copy)
                emit_k_chain(pre2[:, 512:1024], 0, 1, nc.scalar.copy)
                emit_unit(("v", 0))

                # remaining chains with deadlines (emit before attention(d))
                chores.append((1, 860, lambda: emit_unit(("q0r", 0))))
                chores.append((1, 860, lambda: emit_unit(("q0r", 1))))
                for si in range(1, len(kspans)):
                    lo = kspans[si][0] // 128
                    d = next((q for q in range(NQT) if nt_c[q] > lo), NQT - 1)
                    chores.append((d, 860, lambda s=si: emit_unit(("k", s, 0))))
                    chores.append((d, 860, lambda s=si: emit_unit(("k", s, 1))))
                for kb in range(1, nvt):
                    d = next((q for q in range(NQT) if nt_c[q] > kb), NQT - 1)
                    chores.append((d, 860, lambda k=kb: emit_unit(("v", k))))
                for qc in range(1, 4):
                    chores.append((4 * qc, 860, lambda q=qc: emit_unit(("q", q, 0))))
                    chores.append((4 * qc, 860, lambda q=qc: emit_unit(("q", q, 1))))
                chores.sort(key=lambda x: x[0])

                for qt in range(NQT):
                    flush_chores(qt)
                    emit_attention(qt)
                    if qt >= 1:
                        # y-transpose and projection of the previous qtile
                        # ride the chore queue to fill later PE gaps
                        chores.append((qt + (1 if qt < 12 else 2), 150, lambda q=qt - 1: emit_trans(q)))
                        chores.append((qt + 9, 430, lambda q=qt - 1: emit_proj(q)))
                        chores.sort(key=lambda x: x[0])
                flush_chores(10**9)
                emit_trans(NQT - 1)
                emit_proj(NQT - 1)
                assert not chores and not ysb_t and not yts_t, (
                    "unemitted work left behind",
                    len(chores), list(ysb_t), list(yts_t),
                )
    _split_multi_waits(nc, mybir)
    return nc


def _host_inputs(x, mask, Wq, bq, Wk, bk, Wv, bv, Wp, bp, sched):
    """Build the per-core input maps."""
    import ml_dtypes

    bf = ml_dtypes.bfloat16
    f8 = ml_dtypes.float8_e4m3
    it_c, nt_c, nvt, pairs = sched
    naux = len(pairs)
    KW = nvt * 128
    NW = 512 + naux * 128
    scale = 1.0 / math.sqrt(D)

    # query-side penalty pattern aq4 [34, 512]: -PEN where row > lf(q),
    # replicated over the 4 head slots; row 32 = always-mask, row 33 = pad.
    lf = np.arange(128) // NOBJ
    r = np.arange(34)[:, None]
    aq1 = np.where((r > lf[None, :]) & (r < 33), np.float32(-PEN), np.float32(0.0))
    aq4 = np.tile(aq1, (1, 4))  # [34, 512]

    ident = np.eye(128, dtype=np.float32)
    ones_host = np.ones((1, 4 * T), np.float32)

    in_maps = []
    for c in range(NCORES):
        b, g = divmod(c, 2)
        ch = slice(g * 256, (g + 1) * 256)
        vpos = np.nonzero(mask[b])[0]
        nv = len(vpos)
        kf = vpos // NOBJ
        # compacted x for K/V, zero-padded
        xc_h = np.zeros((C, KW), np.float32)
        xc_h[:, :nv] = x[b].T[:, vpos]
        # aux: [34, NW] -> DoubleRow layout [17, 2, NW]
        aux_h = np.zeros((34, NW), np.float32)
        aux_h[:, 0:512] = aq4
        for p, (qt, v) in enumerate(pairs):
            f0 = 32 * qt
            col0 = 512 + p * 128
            for kk in range(128):
                j = v * 128 + kk
                if j >= nv:
                    aux_h[32, col0 + kk] = 1.0  # padding: always masked
                    continue
                i = int(kf[j]) - f0
                if i <= 0:
                    continue  # interior key: no penalty
                aux_h[min(i, 32), col0 + kk] = 1.0
        aux_dr = aux_h.reshape(2, 17, NW).transpose(1, 0, 2).reshape(17, 2 * NW)

        wq_h = np.ascontiguousarray((Wq[ch, :] * scale).T)  # [C, 256]
        wk_h = np.ascontiguousarray(Wk[ch, :].T)
        wv_h = np.ascontiguousarray(Wv[ch, :].T)
        wp_h = np.ascontiguousarray(Wp[:, ch].T)  # [256, 512]
        # pack [128, 4096]: per-kc 256-wide blocks for wq/wk/wv, rc-major wp
        wb_h = np.zeros((128, 4096), np.float32)
        for kc in range(4):
            wb_h[:, kc * 256 : (kc + 1) * 256] = wq_h[kc * 128 : (kc + 1) * 128, :]
            wb_h[:, 1024 + kc * 256 : 1024 + (kc + 1) * 256] = wk_h[
                kc * 128 : (kc + 1) * 128, :
            ]
            wb_h[:, 2048 + kc * 256 : 2048 + (kc + 1) * 256] = wv_h[
                kc * 128 : (kc + 1) * 128, :
            ]
        for rc in range(2):
            wb_h[:, 3072 + rc * 512 : 3072 + (rc + 1) * 512] = wp_h[
                rc * 128 : (rc + 1) * 128, :
            ]

        # kx[h] = xc^T (Wk_h^T (s bq_h)) : folds the Q bias into a 65th K row
        bqs = bq[ch] * scale
        kx_h = np.zeros((4, KW), np.float32)
        for h in range(HPC):
            wkh = Wk[ch, :][h * 64 : (h + 1) * 64, :]  # [64, C]
            w_vec = wkh.T @ bqs[h * 64 : (h + 1) * 64]  # [C]
            kx_h[h] = xc_h.T @ w_vec

        in_maps.append(
            {
                "xt": np.ascontiguousarray(x[b].T).astype(bf),
                "xc": xc_h.astype(bf),
                "wb": wb_h.astype(bf),
                "onesd": ones_host.astype(bf),
                "kxd": kx_h.astype(bf),
                "identd": ident.astype(bf),
                "auxd": aux_dr.astype(f8),
            }
        )
    return in_maps


def kernel(x, mask, Wq, bq, Wk, bk, Wv, bv, Wp, bp):
    from concourse.bass_utils import run_bass_kernel_spmd

    mask = np.asarray(mask)
    sched = _schedule(mask)
    key = ("nc", sched)
    if key not in _CACHE:
        _CACHE[key] = _build_program(*sched)
        _CACHE["nc"] = _CACHE[key]  # for test harness introspection
    nc = _CACHE[key]

    x, Wq, bq, Wk, bk = map(np.asarray, (x, Wq, bq, Wk, bk))
    Wv, bv, Wp, bp = map(np.asarray, (Wv, bv, Wp, bp))
    in_maps = _host_inputs(x, mask, Wq, bq, Wk, bk, Wv, bv, Wp, bp, sched)
    res = run_bass_kernel_spmd(nc, in_maps, core_ids=list(range(NCORES)))
    outs = [res.results[c]["out"] for c in range(NCORES)]
    const = bp + bv @ Wp.T  # bias contributions folded out of the device pass
    y = np.empty((B, T, C), np.float32)
    for b in range(B):
        y[b] = (
            outs[2 * b].astype(np.float32)
            + outs[2 * b + 1].astype(np.float32)
            + const[None, :]
        )
    return y


# revision 94
# speedup vs baseline: 1.0045x; 1.0039x over previous
"""Trainium2 Bass kernel v4: compacted-key block-causal masked attention.

Module: y = proj(softmax(mask(QK^T/sqrt(D))) V) for B=4, T=2048, C=512, H=8,
with a frame-block-causal mask (frame = t//4) and a per-key validity mask.

Sharding: 8 cores = 4 batches x 2 head-groups (4 heads each); host sums the
two partial output projections per batch and adds the projection bias.

v4 restructures v3 around the TimelineSim cost model:
- K bias is dropped entirely (it only shifts each softmax row uniformly) and
  the Q bias is folded into a host-computed 65th K row (kx = xc^T Wk^T bq s)
  against a constant-ones 65th Q row, so Q/K projection chains need plain
  PSUM->SBUF copies instead of bias adds; scores contract over 65 rows,
  which is free on the PE (cost scales with the free dim only).
- The frame-causality penalty for boundary tiles is a single fp8 DoubleRow
  matmul per (qtile, keytile) pair ([17,2]-contraction one-hot against a
  [17,2,512] query-side pattern) - exact, since all values are 0/1/-192.
- V bias and validity-mask multiplies are gone: padding keys are forced to
  exp()=0 by the penalty rows, and bv's contribution is the constant
  bv @ Wp^T added on the host.
- y^T for the output projection comes from PE transpose-matmuls against an
  identity tile instead of serialized DMA transposes; softmax scaling is one
  DVE multiply with a stride-0 broadcast of the reciprocal denominators.
- Output is written bf16 and summed on the host in f32.
- Emission interleaves projection chains, transposes and output projections
  ("chores", output projections deferred up to 9 qtiles) between att tiles
  so the PE always has independent work while an exp() is in flight; input
  DMAs are merged/ordered against the serialized HWDGE (ones/kx rows ride
  the Pool engine's SWDGE path instead).
"""

import math

import numpy as np

B, T, C = 4, 2048, 512
H, NOBJ, D = 8, 4, 64
NCORES = 8
HPC = 4  # heads per core
NQT = 16  # query tiles of 128
BPT = 8  # score blocks per PSUM att tile ([128, 1024] = 2 banks)
PEN = 192.0  # mask penalty (exactly representable in fp8 e4m3)

_CACHE = {}


def _apply_tile_patch(tile_mod, mybir):
    """walrus in this container rejects >1 semaphore wait per instruction;
    spread the TileContext tail-drain waits over sync NOPs (the rest of the
    module is handled by _split_multi_waits after lowering)."""
    import bass_rust

    if getattr(tile_mod.TileContext, "_drain_patched", False):
        return

    def _drain_and_barrier(self, tick_clock, wait_clock):
        nc = self.nc
        drain_inst = nc.sync.drain()
        wait_clock.add_sem_waits(
            drain_inst.ins, bass_rust.ScopedClock({None: tick_clock.global_clock})
        )
        waits = list(drain_inst.ins.sync_info.on_wait or [])
        if len(waits) > 1:
            drain_inst.ins.sync_info.on_wait = []
            for w in waits:
                nop = nc.sync.nop(nofuse=True)
                nop.ins.sync_info = mybir.SyncInfo(on_wait=[w], on_update=[])
            nc.sync.drain()
        nc.all_engine_barrier()
        assert self.sems is not None
        popped = nc._tile_sem_poison_stack.pop()
        assert popped is self._sem_poison
        nc.clear_and_free_semaphores(list(self.sems.allocated().values()))
        nc.all_engine_barrier()

    tile_mod.TileContext._drain_and_barrier = _drain_and_barrier
    tile_mod.TileContext._drain_patched = True


def _split_multi_waits(nc, mybir):
    """Post-pass: for every instruction carrying more than one semaphore
    wait, hoist the extra waits onto same-engine NOPs inserted immediately
    before it (engines execute serially, so blocking at the NOP is
    equivalent)."""
    nonce = 0
    for fn in nc.m.functions:
        for blk in fn.blocks:
            insts = list(blk.instructions)
            out = []
            changed = False
            for ins in insts:
                si = ins.sync_info
                waits = list(si.on_wait) if si and si.on_wait else []
                if len(waits) > 1:
                    changed = True
                    for w in waits[:-1]:
                        nop = mybir.InstNoOp(
                            name=f"I-waitsplit-{nonce}", ins=[], outs=[]
                        )
                        nonce += 1
                        nop.engine = ins.engine
                        nop.sync_info = mybir.SyncInfo(on_wait=[w], on_update=[])
                        nc.register_instruction(nop, overwrite=True)
                        out.append(nop)
                    ins.sync_info.on_wait = waits[-1:]
                out.append(ins)
            if changed:
                blk.instructions = out


def _schedule(mask):
    """Common (max-over-batch) per-qtile interior/total tile counts."""
    it_c, nt_c = [], []
    kfs = []
    for b in range(B):
        vpos = np.nonzero(mask[b])[0]
        kfs.append(vpos // NOBJ)
    for qt in range(NQT):
        f0, flast = 32 * qt, 32 * qt + 31
        its, nts = [], []
        for b in range(B):
            nf = int(np.searchsorted(kfs[b], f0, "right"))
            na = int(np.searchsorted(kfs[b], flast, "right"))
            its.append(nf // 128)
            nts.append(-(-na // 128))
        it_c.append(min(its))
        nt_c.append(max(max(nts), 1))
    nvt = nt_c[-1]
    pairs = []  # (qt, v) -> aux column index order
    for qt in range(NQT):
        for v in range(it_c[qt], nt_c[qt]):
            pairs.append((qt, v))
    return tuple(it_c), tuple(nt_c), nvt, tuple(pairs)


def _kspans(KW):
    """K projection span widths; a short first span unblocks qt0 early."""
    spans = [(0, 128)]
    off = 128
    while off < KW:
        w = min(512, KW - off)
        spans.append((off, w))
        off += w
    return spans


def _build_program(it_c, nt_c, nvt, pairs):
    import concourse.bass as bass
    import concourse.mybir as mybir
    import concourse.tile as tile

    _apply_tile_patch(tile, mybir)

    f32 = mybir.dt.float32
    bf16 = mybir.dt.bfloat16
    fp8 = mybir.dt.float8e4
    EXP = mybir.ActivationFunctionType.Exp
    DR = mybir.MatmulPerfMode.DoubleRow
    naux = len(pairs)
    pair_idx = {p: i for i, p in enumerate(pairs)}
    KW = nvt * 128  # compacted key width
    NW = 512 + naux * 128  # aux free width per half

    nc = bass.Bass(trn_type="TRN2")

    xt = nc.dram_tensor("xt", [C, T], bf16, kind="ExternalInput")
    xc = nc.dram_tensor("xc", [C, KW], bf16, kind="ExternalInput")
    wb = nc.dram_tensor("wb", [128, 4096], bf16, kind="ExternalInput")
    onesd = nc.dram_tensor("onesd", [1, 4 * T], bf16, kind="ExternalInput")
    kxd = nc.dram_tensor("kxd", [4, KW], bf16, kind="ExternalInput")
    identd = nc.dram_tensor("identd", [128, 128], bf16, kind="ExternalInput")
    auxd = nc.dram_tensor("auxd", [17, 2 * NW], fp8, kind="ExternalInput")
    out = nc.dram_tensor("out", [T, C], bf16, kind="ExternalOutput")

    def mm(o, lhsT, rhs, start, stop, **kw):
        nc.tensor.matmul(o, lhsT, rhs, start=start, stop=stop, **kw)

    kspans = _kspans(KW)

    with nc.allow_low_precision(
        reason="bf16/fp8 matmul inputs; PSUM accumulation stays fp32"
    ), tile.TileContext(nc) as tc:
        with tc.tile_pool(name="const", bufs=1) as cp:
            w_s = cp.tile([128, 4096], bf16)
            xts = cp.tile([128, 4, T], bf16)
            xcs = cp.tile([128, 4, KW], bf16)
            ident = cp.tile([128, 128], bf16)
            aux_s = cp.tile([17, 2, NW], fp8)
            zer_s = cp.tile([1, 128], bf16)
            qtd = cp.tile([65, 4 * T], bf16)
            ktd = cp.tile([65, 4 * KW], bf16)
            v4 = cp.tile([128, nvt, 4, 65], bf16)

            wq_s = w_s[:, 0:1024]
            wk_s = w_s[:, 1024:2048]
            wv_s = w_s[:, 2048:3072]
            wp_s = w_s[:, 3072:4096]

            # ---- input DMAs, ordered by first use (HWDGE serializes) ----
            xtv = xt[:].rearrange("(k p) t -> p k t", p=128)
            xcv = xc[:].rearrange("(k p) t -> p k t", p=128)
            nc.gpsimd.dma_start(ktd[64:65, :], kxd[:])
            nc.gpsimd.dma_start(qtd[64:65, :], onesd[:])
            nc.sync.dma_start(w_s[:, 0:1024], wb[:, 0:1024])
            nc.sync.dma_start(xts[:, :, 0:128], xtv[:, :, 0:128])
            nc.sync.dma_start(xcs[:, :, 0:128], xcv[:, :, 0:128])
            nc.sync.dma_start(w_s[:, 1024:2048], wb[:, 1024:2048])
            nc.sync.dma_start(w_s[:, 2048:3072], wb[:, 2048:3072])
            nc.sync.dma_start(aux_s[:, :, :], auxd[:])
            nc.sync.dma_start(xts[:, :, 128:512], xtv[:, :, 128:512])
            nc.sync.dma_start(xcs[:, :, 128:640], xcv[:, :, 128:640])
            nc.sync.dma_start(xcs[:, :, 640:KW], xcv[:, :, 640:KW])
            nc.sync.dma_start(ident[:], identd[:])
            nc.sync.dma_start(w_s[:, 3072:4096], wb[:, 3072:4096])
            nc.sync.dma_start(xts[:, :, 512:1280], xtv[:, :, 512:1280])
            nc.sync.dma_start(xts[:, :, 1280:2048], xtv[:, :, 1280:2048])
            nc.vector.memset(zer_s[:], 0.0)
            # denominator columns of v4 are constant 1
            nc.vector.memset(v4[:, :, :, 64:65], 1.0)

            with tc.tile_pool(name="attp", bufs=2, space="PSUM") as attp, tc.tile_pool(
                name="yaccp", bufs=2, space="PSUM"
            ) as yaccp, tc.tile_pool(
                name="smp", bufs=2, space="PSUM"
            ) as smp, tc.tile_pool(name="ptp", bufs=3) as ptp, tc.tile_pool(
                name="ysbp", bufs=4
            ) as ysbp, tc.tile_pool(name="rrp", bufs=4) as rrp, tc.tile_pool(
                name="ytsp", bufs=12
            ) as ytsp, tc.tile_pool(name="osp", bufs=12) as osp:
                nonce = [0]

                def emit_q_chain(ps, qc, rc, copy_eng, c0=0, c1=512):
                    w = c1 - c0
                    for kc in range(4):
                        mm(
                            ps[:, 0:w],
                            wq_s[:, kc * 256 + rc * 128 : kc * 256 + rc * 128 + 128],
                            xts[:, kc, qc * 512 + c0 : qc * 512 + c1],
                            kc == 0,
                            kc == 3,
                        )
                    for hh in range(2):
                        h = 2 * rc + hh
                        copy_eng(
                            qtd[0:64, h * T + qc * 512 + c0 : h * T + qc * 512 + c1],
                            ps[hh * 64 : (hh + 1) * 64, 0:w],
                        )

                def emit_k_chain(ps, si, rc, copy_eng):
                    off, w = kspans[si]
                    for kc in range(4):
                        mm(
                            ps[:, 0:w],
                            wk_s[:, kc * 256 + rc * 128 : kc * 256 + rc * 128 + 128],
                            xcs[:, kc, off : off + w],
                            kc == 0,
                            kc == 3,
                        )
                    for hh in range(2):
                        h = 2 * rc + hh
                        copy_eng(
                            ktd[0:64, h * KW + off : h * KW + off + w],
                            ps[hh * 64 : (hh + 1) * 64, 0:w],
                        )

                def emit_v_chain(ps, kb, copy_eng):
                    for kc in range(4):
                        mm(
                            ps[:, 0:256],
                            xcs[:, kc, kb * 128 : kb * 128 + 128],
                            wv_s[:, kc * 256 : (kc + 1) * 256],
                            kc == 0,
                            kc == 3,
                        )
                    copy_eng(v4[:, kb, :, 0:64], ps[:, 0:256])

                def emit_unit(u, ps=None):
                    if ps is None:
                        ps = smp.tile([128, 512], f32, tag="sm", name=f"u{nonce[0]}")
                        nonce[0] += 1
                    kind = u[0]
                    if kind == "q":
                        emit_q_chain(ps, u[1], u[2], nc.vector.tensor_copy)
                    elif kind == "q0r":
                        emit_q_chain(ps, 0, u[1], nc.vector.tensor_copy, 128, 512)
                    elif kind == "k":
                        emit_k_chain(ps, u[1], u[2], nc.vector.tensor_copy)
                    else:
                        emit_v_chain(ps, u[1], nc.vector.tensor_copy)

                def qt_tiles(qt):
                    """Slot items per att tile: interior head-blocks first,
                    then 4-aligned boundary groups."""
                    items = [("blk", h, kb) for h in range(HPC) for kb in range(it_c[qt])]
                    for v in range(it_c[qt], nt_c[qt]):
                        items.append(("grp", v))
                    tiles = []
                    cur, used = [], 0
                    for itm in items:
                        sz = 4 if itm[0] == "grp" else 1
                        if used + sz > BPT:
                            tiles.append((cur, used))
                            cur, used = [], 0
                        cur.append((used, itm))
                        used += sz
                    if cur:
                        tiles.append((cur, used))
                    return tiles

                def emit_scores(qt, at, take):
                    for slot, itm in take:
                        if itm[0] == "blk":
                            _, h, kb = itm
                            mm(
                                at[:, slot * 128 : (slot + 1) * 128],
                                ktd[0:65, h * KW + kb * 128 : h * KW + (kb + 1) * 128],
                                qtd[0:65, h * T + qt * 128 : h * T + (qt + 1) * 128],
                                True,
                                True,
                            )
                        else:
                            v = itm[1]
                            for hh in range(HPC):
                                mm(
                                    at[:, (slot + hh) * 128 : (slot + hh + 1) * 128],
                                    ktd[0:65, hh * KW + v * 128 : hh * KW + (v + 1) * 128],
                                    qtd[0:65, hh * T + qt * 128 : hh * T + (qt + 1) * 128],
                                    hh == 0,
                                    False,
                                    skip_group_check=True,
                                )
                            p = pair_idx[(qt, v)]
                            mm(
                                at[:, slot * 128 : (slot + 4) * 128],
                                aux_s[:, :, 512 + p * 128 : 512 + (p + 1) * 128],
                                aux_s[:, :, 0:512],
                                False,
                                True,
                                perf_mode=DR,
                                skip_group_check=True,
                            )

                def emit_pv_tile(qt, yacc, pt, take):
                    nt = nt_c[qt]
                    for slot, itm in take:
                        if itm[0] == "blk":
                            blocks = [(slot, itm[1], itm[2])]
                        else:
                            blocks = [(slot + hh, hh, itm[1]) for hh in range(HPC)]
                        for sl, h, kb in blocks:
                            mm(
                                yacc[:, h * 65 : h * 65 + 65],
                                pt[:, sl * 128 : (sl + 1) * 128],
                                v4[:, kb, h, :],
                                False,
                                kb == nt - 1,
                                skip_group_check=True,
                            )

                ysb_t = {}
                yts_t = {}

                def emit_fin(qt, yacc):
                    """Recip the denominators, scale all heads into bf16 with
                    one broadcast multiply."""
                    rr = rrp.tile([128, 4], f32, tag="rr", name=f"rr{qt}")
                    nc.vector.reciprocal(rr[:], yacc[:, 64:260:65])
                    ysb = ysbp.tile([128, 256], bf16, tag="ysb", name=f"ysb{qt}")
                    yv = yacc.rearrange("p (h d) -> p h d", h=4)[:, :, 0:64]
                    nc.vector.tensor_mul(
                        ysb[:].rearrange("p (h d) -> p h d", h=4),
                        yv,
                        rr[:].unsqueeze(2).broadcast_to([128, 4, 64]),
                    )
                    ysb_t[qt] = ysb

                def emit_trans(qt):
                    ysb = ysb_t.pop(qt)
                    ytp = yaccp.tile([128, 256], bf16, tag="yacc", name=f"ytp{qt}")
                    nc.tensor.transpose(ytp[:, 0:128], ysb[:, 0:128], ident[:])
                    nc.tensor.transpose(ytp[:, 128:256], ysb[:, 128:256], ident[:])
                    yts = ytsp.tile([128, 256], bf16, tag="yts", name=f"yts{qt}")
                    nc.vector.tensor_copy(yts[:], ytp[:])
                    yts_t[qt] = yts

                def emit_proj(qt):
                    yts = yts_t.pop(qt)
                    po = smp.tile([128, 512], f32, tag="sm", name=f"po{qt}")
                    mm(po[:], yts[:, 0:128], wp_s[:, 0:512], True, False)
                    mm(po[:], yts[:, 128:256], wp_s[:, 512:1024], False, True)
                    os_t = osp.tile([128, 512], bf16, tag="os", name=f"os{qt}")
                    nc.vector.tensor_copy(os_t[:], po[:])
                    nc.sync.dma_start(out[qt * 128 : (qt + 1) * 128, :], os_t[:])

                # chores: (deadline qt, pe_ns, thunk) consumed between att
                # tiles so the PE always has independent work while exp()
                # runs; pop ~a tile-period's worth of PE work each time.
                chores = []

                def pop_chore(qt):
                    if chores and chores[0][0] <= qt + 1:
                        chores.pop(0)[2]()

                def flush_chores(qt):
                    while chores and chores[0][0] <= qt:
                        chores.pop(0)[2]()

                def emit_attention(qt):
                    tiles = qt_tiles(qt)
                    yacc = yaccp.tile([128, 260], f32, tag="yacc", name=f"yacc{qt}")
                    ptiles = []
                    for t, (tb, used) in enumerate(tiles):
                        at = attp.tile([128, BPT * 128], f32, tag="att", name=f"at{qt}_{t}")
                        emit_scores(qt, at, tb)
                        if t == 0:
                            # zero-init the accumulator bank with a 1-row
                            # matmul, deferred past the first scores so it
                            # never head-blocks the PE at qtile start
                            mm(yacc[:, 0:260], zer_s[:], zbias_s[:, 0:260],
                               True, False, skip_group_check=True)
                        pt = ptp.tile([128, BPT * 128], bf16, tag="p", name=f"p{qt}_{t}")
                        nc.scalar.activation(pt[:, 0 : used * 128], at[:, 0 : used * 128], EXP)
                        ptiles.append((pt, tb))
                        pop_chore(qt)
                        if t >= 1:
                            emit_pv_tile(qt, yacc, *ptiles[t - 1])
                    emit_pv_tile(qt, yacc, *ptiles[-1])
                    emit_fin(qt, yacc)
                    pop_chore(qt)

                zbias_s = cp.tile([1, 260], bf16)
                nc.vector.memset(zbias_s[:], 0.0)

                # ---- prelude: chains needed by the first query tiles ----
                # park early chains in the idle att-psum banks to run them
                # in parallel; the rest flow through the sm pool.
                pre1 = attp.tile([128, BPT * 128], f32, tag="att", name="pre1")
                emit_q_chain(pre1[:, 0:512], 0, 0, nc.vector.tensor_copy, 0, 128)
                pre2 = attp.tile([128, BPT * 128], f32, tag="att", name="pre2")
                emit_q_chain(pre2[:, 0:512], 0, 1, nc.vector.tensor_copy, 0, 128)
                emit_k_chain(pre1[:, 512:1024], 0, 0, nc.scalar.copy)
                emit_k_chain(pre2[:, 512:1024], 0, 1, nc.scalar.copy)
                emit_unit(("v", 0))

                # remaining chains with deadlines (emit before attention(d))
                chores.append((1, 860, lambda: emit_unit(("q0r", 0))))
                chores.append((1, 860, lambda: emit_unit(("q0r", 1))))
                for si in range(1, len(kspans)):
                    lo = kspans[si][0] // 128
                    d = next((q for q in range(NQT) if nt_c[q] > lo), NQT - 1)
                    chores.append((d, 860, lambda s=si: emit_unit(("k", s, 0))))
                    chores.append((d, 860, lambda s=si: emit_unit(("k", s, 1))))
                for kb in range(1, nvt):
                    d = next((q for q in range(NQT) if nt_c[q] > kb), NQT - 1)
                    chores.append((d, 860, lambda k=kb: emit_unit(("v", k))))
                for qc in range(1, 4):
                    chores.append((4 * qc, 860, lambda q=qc: emit_unit(("q", q, 0))))
                    chores.append((4 * qc, 860, lambda q=qc: emit_unit(("q", q, 1))))
                chores.sort(key=lambda x: x[0])

                for qt in range(NQT):
                    flush_chores(qt)
                    emit_attention(qt)
                    if qt >= 1:
                        # y-transpose and projection of the previous qtile
                        # ride the chore queue to fill later PE gaps
                        chores.append((qt + (1 if qt < 12 else 2), 150, lambda q=qt - 1: emit_trans(q)))
                        chores.append((qt + 12, 430, lambda q=qt - 1: emit_proj(q)))
                        chores.sort(key=lambda x: x[0])
                flush_chores(10**9)
                emit_trans(NQT - 1)
                emit_proj(NQT - 1)
                assert not chores and not ysb_t and not yts_t, (
                    "unemitted work left behind",
                    len(chores), list(ysb_t), list(yts_t),
                )
    _split_multi_waits(nc, mybir)
    return nc


def _host_inputs(x, mask, Wq, bq, Wk, bk, Wv, bv, Wp, bp, sched):
    """Build the per-core input maps."""
    import ml_dtypes

    bf = ml_dtypes.bfloat16
    f8 = ml_dtypes.float8_e4m3
    it_c, nt_c, nvt, pairs = sched
    naux = len(pairs)
    KW = nvt * 128
    NW = 512 + naux * 128
    scale = 1.0 / math.sqrt(D)

    # query-side penalty pattern aq4 [34, 512]: -PEN where row > lf(q),
    # replicated over the 4 head slots; row 32 = always-mask, row 33 = pad.
    lf = np.arange(128) // NOBJ
    r = np.arange(34)[:, None]
    aq1 = np.where((r > lf[None, :]) & (r < 33), np.float32(-PEN), np.float32(0.0))
    aq4 = np.tile(aq1, (1, 4))  # [34, 512]

    ident = np.eye(128, dtype=np.float32)
    ones_host = np.ones((1, 4 * T), np.float32)

    in_maps = []
    for c in range(NCORES):
        b, g = divmod(c, 2)
        ch = slice(g * 256, (g + 1) * 256)
        vpos = np.nonzero(mask[b])[0]
        nv = len(vpos)
        kf = vpos // NOBJ
        # compacted x for K/V, zero-padded
        xc_h = np.zeros((C, KW), np.float32)
        xc_h[:, :nv] = x[b].T[:, vpos]
        # aux: [34, NW] -> DoubleRow layout [17, 2, NW]
        aux_h = np.zeros((34, NW), np.float32)
        aux_h[:, 0:512] = aq4
        for p, (qt, v) in enumerate(pairs):
            f0 = 32 * qt
            col0 = 512 + p * 128
            for kk in range(128):
                j = v * 128 + kk
                if j >= nv:
                    aux_h[32, col0 + kk] = 1.0  # padding: always masked
                    continue
                i = int(kf[j]) - f0
                if i <= 0:
                    continue  # interior key: no penalty
                aux_h[min(i, 32), col0 + kk] = 1.0
        aux_dr = aux_h.reshape(2, 17, NW).transpose(1, 0, 2).reshape(17, 2 * NW)

        wq_h = np.ascontiguousarray((Wq[ch, :] * scale).T)  # [C, 256]
        wk_h = np.ascontiguousarray(Wk[ch, :].T)
        wv_h = np.ascontiguousarray(Wv[ch, :].T)
        wp_h = np.ascontiguousarray(Wp[:, ch].T)  # [256, 512]
        # pack [128, 4096]: per-kc 256-wide blocks for wq/wk/wv, rc-major wp
        wb_h = np.zeros((128, 4096), np.float32)
        for kc in range(4):
            wb_h[:, kc * 256 : (kc + 1) * 256] = wq_h[kc * 128 : (kc + 1) * 128, :]
            wb_h[:, 1024 + kc * 256 : 1024 + (kc + 1) * 256] = wk_h[
                kc * 128 : (kc + 1) * 128, :
            ]
            wb_h[:, 2048 + kc * 256 : 2048 + (kc + 1) * 256] = wv_h[
                kc * 128 : (kc + 1) * 128, :
            ]
        for rc in range(2):
            wb_h[:, 3072 + rc * 512 : 3072 + (rc + 1) * 512] = wp_h[
                rc * 128 : (rc + 1) * 128, :
            ]

        # kx[h] = xc^T (Wk_h^T (s bq_h)) : folds the Q bias into a 65th K row
        bqs = bq[ch] * scale
        kx_h = np.zeros((4, KW), np.float32)
        for h in range(HPC):
            wkh = Wk[ch, :][h * 64 : (h + 1) * 64, :]  # [64, C]
            w_vec = wkh.T @ bqs[h * 64 : (h + 1) * 64]  # [C]
            kx_h[h] = xc_h.T @ w_vec

        in_maps.append(
            {
                "xt": np.ascontiguousarray(x[b].T).astype(bf),
                "xc": xc_h.astype(bf),
                "wb": wb_h.astype(bf),
                "onesd": ones_host.astype(bf),
                "kxd": kx_h.astype(bf),
                "identd": ident.astype(bf),
                "auxd": aux_dr.astype(f8),
            }
        )
    return in_maps


def kernel(x, mask, Wq, bq, Wk, bk, Wv, bv, Wp, bp):
    from concourse.bass_utils import run_bass_kernel_spmd

    mask = np.asarray(mask)
    sched = _schedule(mask)
    key = ("nc", sched)
    if key not in _CACHE:
        _CACHE[key] = _build_program(*sched)
        _CACHE["nc"] = _CACHE[key]  # for test harness introspection
    nc = _CACHE[key]

    x, Wq, bq, Wk, bk = map(np.asarray, (x, Wq, bq, Wk, bk))
    Wv, bv, Wp, bp = map(np.asarray, (Wv, bv, Wp, bp))
    in_maps = _host_inputs(x, mask, Wq, bq, Wk, bk, Wv, bv, Wp, bp, sched)
    res = run_bass_kernel_spmd(nc, in_maps, core_ids=list(range(NCORES)))
    outs = [res.results[c]["out"] for c in range(NCORES)]
    const = bp + bv @ Wp.T  # bias contributions folded out of the device pass
    y = np.empty((B, T, C), np.float32)
    for b in range(B):
        y[b] = (
            outs[2 * b].astype(np.float32)
            + outs[2 * b + 1].astype(np.float32)
            + const[None, :]
        )
    return y


# revision 101
# speedup vs baseline: 1.0048x; 1.0004x over previous
"""Trainium2 Bass kernel v4: compacted-key block-causal masked attention.

Module: y = proj(softmax(mask(QK^T/sqrt(D))) V) for B=4, T=2048, C=512, H=8,
with a frame-block-causal mask (frame = t//4) and a per-key validity mask.

Sharding: 8 cores = 4 batches x 2 head-groups (4 heads each); host sums the
two partial output projections per batch and adds the projection bias.

v4 restructures v3 around the TimelineSim cost model:
- K bias is dropped entirely (it only shifts each softmax row uniformly) and
  the Q bias is folded into a host-computed 65th K row (kx = xc^T Wk^T bq s)
  against a constant-ones 65th Q row, so Q/K projection chains need plain
  PSUM->SBUF copies instead of bias adds; scores contract over 65 rows,
  which is free on the PE (cost scales with the free dim only).
- The frame-causality penalty for boundary tiles is a single fp8 DoubleRow
  matmul per (qtile, keytile) pair ([17,2]-contraction one-hot against a
  [17,2,512] query-side pattern) - exact, since all values are 0/1/-192.
- V bias and validity-mask multiplies are gone: padding keys are forced to
  exp()=0 by the penalty rows, and bv's contribution is the constant
  bv @ Wp^T added on the host.
- y^T for the output projection comes from PE transpose-matmuls against an
  identity tile instead of serialized DMA transposes; softmax scaling is one
  DVE multiply with a stride-0 broadcast of the reciprocal denominators.
- Output is written bf16 and summed on the host in f32.
- Emission interleaves projection chains, transposes and output projections
  ("chores", output projections deferred up to 9 qtiles) between att tiles
  so the PE always has independent work while an exp() is in flight; input
  DMAs are merged/ordered against the serialized HWDGE (ones/kx rows ride
  the Pool engine's SWDGE path instead).
"""

import math

import numpy as np

B, T, C = 4, 2048, 512
H, NOBJ, D = 8, 4, 64
NCORES = 8
HPC = 4  # heads per core
NQT = 16  # query tiles of 128
BPT = 8  # score blocks per PSUM att tile ([128, 1024] = 2 banks)
PEN = 192.0  # mask penalty (exactly representable in fp8 e4m3)

_CACHE = {}


def _apply_tile_patch(tile_mod, mybir):
    """walrus in this container rejects >1 semaphore wait per instruction;
    spread the TileContext tail-drain waits over sync NOPs (the rest of the
    module is handled by _split_multi_waits after lowering)."""
    import bass_rust

    if getattr(tile_mod.TileContext, "_drain_patched", False):
        return

    def _drain_and_barrier(self, tick_clock, wait_clock):
        nc = self.nc
        drain_inst = nc.sync.drain()
        wait_clock.add_sem_waits(
            drain_inst.ins, bass_rust.ScopedClock({None: tick_clock.global_clock})
        )
        waits = list(drain_inst.ins.sync_info.on_wait or [])
        if len(waits) > 1:
            drain_inst.ins.sync_info.on_wait = []
            for w in waits:
                nop = nc.sync.nop(nofuse=True)
                nop.ins.sync_info = mybir.SyncInfo(on_wait=[w], on_update=[])
            nc.sync.drain()
        nc.all_engine_barrier()
        assert self.sems is not None
        popped = nc._tile_sem_poison_stack.pop()
        assert popped is self._sem_poison
        nc.clear_and_free_semaphores(list(self.sems.allocated().values()))
        nc.all_engine_barrier()

    tile_mod.TileContext._drain_and_barrier = _drain_and_barrier
    tile_mod.TileContext._drain_patched = True


def _split_multi_waits(nc, mybir):
    """Post-pass: for every instruction carrying more than one semaphore
    wait, hoist the extra waits onto same-engine NOPs inserted immediately
    before it (engines execute serially, so blocking at the NOP is
    equivalent)."""
    nonce = 0
    for fn in nc.m.functions:
        for blk in fn.blocks:
            insts = list(blk.instructions)
            out = []
            changed = False
            for ins in insts:
                si = ins.sync_info
                waits = list(si.on_wait) if si and si.on_wait else []
                if len(waits) > 1:
                    changed = True
                    for w in waits[:-1]:
                        nop = mybir.InstNoOp(
                            name=f"I-waitsplit-{nonce}", ins=[], outs=[]
                        )
                        nonce += 1
                        nop.engine = ins.engine
                        nop.sync_info = mybir.SyncInfo(on_wait=[w], on_update=[])
                        nc.register_instruction(nop, overwrite=True)
                        out.append(nop)
                    ins.sync_info.on_wait = waits[-1:]
                out.append(ins)
            if changed:
                blk.instructions = out


def _schedule(mask):
    """Common (max-over-batch) per-qtile interior/total tile counts."""
    it_c, nt_c = [], []
    kfs = []
    for b in range(B):
        vpos = np.nonzero(mask[b])[0]
        kfs.append(vpos // NOBJ)
    for qt in range(NQT):
        f0, flast = 32 * qt, 32 * qt + 31
        its, nts = [], []
        for b in range(B):
            nf = int(np.searchsorted(kfs[b], f0, "right"))
            na = int(np.searchsorted(kfs[b], flast, "right"))
            its.append(nf // 128)
            nts.append(-(-na // 128))
        it_c.append(min(its))
        nt_c.append(max(max(nts), 1))
    nvt = nt_c[-1]
    pairs = []  # (qt, v) -> aux column index order
    for qt in range(NQT):
        for v in range(it_c[qt], nt_c[qt]):
            pairs.append((qt, v))
    return tuple(it_c), tuple(nt_c), nvt, tuple(pairs)


def _kspans(KW):
    """K projection span widths; a short first span unblocks qt0 early."""
    spans = [(0, 128)]
    off = 128
    while off < KW:
        w = min(512, KW - off)
        spans.append((off, w))
        off += w
    return spans


def _build_program(it_c, nt_c, nvt, pairs):
    import concourse.bass as bass
    import concourse.mybir as mybir
    import concourse.tile as tile

    _apply_tile_patch(tile, mybir)

    f32 = mybir.dt.float32
    bf16 = mybir.dt.bfloat16
    fp8 = mybir.dt.float8e4
    EXP = mybir.ActivationFunctionType.Exp
    DR = mybir.MatmulPerfMode.DoubleRow
    naux = len(pairs)
    pair_idx = {p: i for i, p in enumerate(pairs)}
    KW = nvt * 128  # compacted key width
    NW = 512 + naux * 128  # aux free width per half

    nc = bass.Bass(trn_type="TRN2")

    xt = nc.dram_tensor("xt", [C, T], bf16, kind="ExternalInput")
    xc = nc.dram_tensor("xc", [C, KW], bf16, kind="ExternalInput")
    wb = nc.dram_tensor("wb", [128, 4096], bf16, kind="ExternalInput")
    onesd = nc.dram_tensor("onesd", [1, 4 * T], bf16, kind="ExternalInput")
    kxd = nc.dram_tensor("kxd", [4, KW], bf16, kind="ExternalInput")
    identd = nc.dram_tensor("identd", [128, 128], bf16, kind="ExternalInput")
    auxd = nc.dram_tensor("auxd", [17, 2 * NW], fp8, kind="ExternalInput")
    out = nc.dram_tensor("out", [T, C], bf16, kind="ExternalOutput")

    def mm(o, lhsT, rhs, start, stop, **kw):
        nc.tensor.matmul(o, lhsT, rhs, start=start, stop=stop, **kw)

    kspans = _kspans(KW)

    with nc.allow_low_precision(
        reason="bf16/fp8 matmul inputs; PSUM accumulation stays fp32"
    ), tile.TileContext(nc) as tc:
        with tc.tile_pool(name="const", bufs=1) as cp:
            w_s = cp.tile([128, 4096], bf16)
            xts = cp.tile([128, 4, T], bf16)
            xcs = cp.tile([128, 4, KW], bf16)
            ident = cp.tile([128, 128], bf16)
            aux_s = cp.tile([17, 2, NW], fp8)
            zer_s = cp.tile([1, 128], bf16)
            qtd = cp.tile([65, 4 * T], bf16)
            ktd = cp.tile([65, 4 * KW], bf16)
            v4 = cp.tile([128, nvt, 4, 65], bf16)

            wq_s = w_s[:, 0:1024]
            wk_s = w_s[:, 1024:2048]
            wv_s = w_s[:, 2048:3072]
            wp_s = w_s[:, 3072:4096]

            # ---- input DMAs, ordered by first use (HWDGE serializes) ----
            xtv = xt[:].rearrange("(k p) t -> p k t", p=128)
            xcv = xc[:].rearrange("(k p) t -> p k t", p=128)
            nc.gpsimd.dma_start(ktd[64:65, :], kxd[:])
            nc.gpsimd.dma_start(qtd[64:65, :], onesd[:])
            nc.sync.dma_start(w_s[:, 0:1024], wb[:, 0:1024])
            nc.sync.dma_start(xts[:, :, 0:128], xtv[:, :, 0:128])
            nc.sync.dma_start(xcs[:, :, 0:128], xcv[:, :, 0:128])
            nc.sync.dma_start(w_s[:, 1024:2048], wb[:, 1024:2048])
            nc.sync.dma_start(w_s[:, 2048:3072], wb[:, 2048:3072])
            nc.sync.dma_start(aux_s[:, :, :], auxd[:])
            nc.sync.dma_start(xts[:, :, 128:512], xtv[:, :, 128:512])
            nc.sync.dma_start(xcs[:, :, 128:640], xcv[:, :, 128:640])
            nc.sync.dma_start(xcs[:, :, 640:KW], xcv[:, :, 640:KW])
            nc.sync.dma_start(ident[:], identd[:])
            nc.sync.dma_start(w_s[:, 3072:4096], wb[:, 3072:4096])
            nc.sync.dma_start(xts[:, :, 512:1280], xtv[:, :, 512:1280])
            nc.sync.dma_start(xts[:, :, 1280:2048], xtv[:, :, 1280:2048])
            nc.vector.memset(zer_s[:], 0.0)
            # denominator columns of v4 are constant 1
            nc.vector.memset(v4[:, :, :, 64:65], 1.0)

            with tc.tile_pool(name="attp", bufs=2, space="PSUM") as attp, tc.tile_pool(
                name="yaccp", bufs=2, space="PSUM"
            ) as yaccp, tc.tile_pool(
                name="smp", bufs=2, space="PSUM"
            ) as smp, tc.tile_pool(name="ptp", bufs=3) as ptp, tc.tile_pool(
                name="ysbp", bufs=4
            ) as ysbp, tc.tile_pool(name="rrp", bufs=4) as rrp, tc.tile_pool(
                name="ytsp", bufs=12
            ) as ytsp, tc.tile_pool(name="osp", bufs=12) as osp:
                nonce = [0]

                def emit_q_chain(ps, qc, rc, copy_eng, c0=0, c1=512):
                    w = c1 - c0
                    for kc in range(4):
                        mm(
                            ps[:, 0:w],
                            wq_s[:, kc * 256 + rc * 128 : kc * 256 + rc * 128 + 128],
                            xts[:, kc, qc * 512 + c0 : qc * 512 + c1],
                            kc == 0,
                            kc == 3,
                        )
                    for hh in range(2):
                        h = 2 * rc + hh
                        copy_eng(
                            qtd[0:64, h * T + qc * 512 + c0 : h * T + qc * 512 + c1],
                            ps[hh * 64 : (hh + 1) * 64, 0:w],
                        )

                def emit_k_chain(ps, si, rc, copy_eng):
                    off, w = kspans[si]
                    for kc in range(4):
                        mm(
                            ps[:, 0:w],
                            wk_s[:, kc * 256 + rc * 128 : kc * 256 + rc * 128 + 128],
                            xcs[:, kc, off : off + w],
                            kc == 0,
                            kc == 3,
                        )
                    for hh in range(2):
                        h = 2 * rc + hh
                        copy_eng(
                            ktd[0:64, h * KW + off : h * KW + off + w],
                            ps[hh * 64 : (hh + 1) * 64, 0:w],
                        )

                def emit_v_chain(ps, kb, copy_eng):
                    for kc in range(4):
                        mm(
                            ps[:, 0:256],
                            xcs[:, kc, kb * 128 : kb * 128 + 128],
                            wv_s[:, kc * 256 : (kc + 1) * 256],
                            kc == 0,
                            kc == 3,
                        )
                    copy_eng(v4[:, kb, :, 0:64], ps[:, 0:256])

                def emit_unit(u, ps=None):
                    if ps is None:
                        ps = smp.tile([128, 512], f32, tag="sm", name=f"u{nonce[0]}")
                        nonce[0] += 1
                    kind = u[0]
                    if kind == "q":
                        emit_q_chain(ps, u[1], u[2], nc.vector.tensor_copy)
                    elif kind == "q0r":
                        emit_q_chain(ps, 0, u[1], nc.vector.tensor_copy, 128, 512)
                    elif kind == "k":
                        emit_k_chain(ps, u[1], u[2], nc.vector.tensor_copy)
                    else:
                        emit_v_chain(ps, u[1], nc.vector.tensor_copy)

                def qt_tiles(qt):
                    """Slot items per att tile: interior head-blocks first,
                    then 4-aligned boundary groups."""
                    items = [("blk", h, kb) for h in range(HPC) for kb in range(it_c[qt])]
                    for v in range(it_c[qt], nt_c[qt]):
                        items.append(("grp", v))
                    tiles = []
                    cur, used = [], 0
                    for itm in items:
                        sz = 4 if itm[0] == "grp" else 1
                        if used + sz > BPT:
                            tiles.append((cur, used))
                            cur, used = [], 0
                        cur.append((used, itm))
                        used += sz
                    if cur:
                        tiles.append((cur, used))
                    return tiles

                def emit_scores(qt, at, take):
                    for slot, itm in take:
                        if itm[0] == "blk":
                            _, h, kb = itm
                            mm(
                                at[:, slot * 128 : (slot + 1) * 128],
                                ktd[0:65, h * KW + kb * 128 : h * KW + (kb + 1) * 128],
                                qtd[0:65, h * T + qt * 128 : h * T + (qt + 1) * 128],
                                True,
                                True,
                            )
                        else:
                            v = itm[1]
                            for hh in range(HPC):
                                mm(
                                    at[:, (slot + hh) * 128 : (slot + hh + 1) * 128],
                                    ktd[0:65, hh * KW + v * 128 : hh * KW + (v + 1) * 128],
                                    qtd[0:65, hh * T + qt * 128 : hh * T + (qt + 1) * 128],
                                    hh == 0,
                                    False,
                                    skip_group_check=True,
                                )
                            p = pair_idx[(qt, v)]
                            mm(
                                at[:, slot * 128 : (slot + 4) * 128],
                                aux_s[:, :, 512 + p * 128 : 512 + (p + 1) * 128],
                                aux_s[:, :, 0:512],
                                False,
                                True,
                                perf_mode=DR,
                                skip_group_check=True,
                            )

                def emit_pv_tile(qt, yacc, pt, take):
                    nt = nt_c[qt]
                    for slot, itm in take:
                        if itm[0] == "blk":
                            blocks = [(slot, itm[1], itm[2])]
                        else:
                            blocks = [(slot + hh, hh, itm[1]) for hh in range(HPC)]
                        for sl, h, kb in blocks:
                            mm(
                                yacc[:, h * 65 : h * 65 + 65],
                                pt[:, sl * 128 : (sl + 1) * 128],
                                v4[:, kb, h, :],
                                False,
                                kb == nt - 1,
                                skip_group_check=True,
                            )

                ysb_t = {}
                yts_t = {}

                def emit_fin(qt, yacc):
                    """Recip the denominators, scale all heads into bf16 with
                    one broadcast multiply."""
                    rr = rrp.tile([128, 4], f32, tag="rr", name=f"rr{qt}")
                    nc.vector.reciprocal(rr[:], yacc[:, 64:260:65])
                    ysb = ysbp.tile([128, 256], bf16, tag="ysb", name=f"ysb{qt}")
                    yv = yacc.rearrange("p (h d) -> p h d", h=4)[:, :, 0:64]
                    nc.vector.tensor_mul(
                        ysb[:].rearrange("p (h d) -> p h d", h=4),
                        yv,
                        rr[:].unsqueeze(2).broadcast_to([128, 4, 64]),
                    )
                    ysb_t[qt] = ysb

                def emit_trans(qt):
                    ysb = ysb_t.pop(qt)
                    ytp = yaccp.tile([128, 256], bf16, tag="yacc", name=f"ytp{qt}")
                    nc.tensor.transpose(ytp[:, 0:128], ysb[:, 0:128], ident[:])
                    nc.tensor.transpose(ytp[:, 128:256], ysb[:, 128:256], ident[:])
                    yts = ytsp.tile([128, 256], bf16, tag="yts", name=f"yts{qt}")
                    nc.vector.tensor_copy(yts[:], ytp[:])
                    yts_t[qt] = yts

                def emit_proj(qt):
                    yts = yts_t.pop(qt)
                    po = smp.tile([128, 512], f32, tag="sm", name=f"po{qt}")
                    mm(po[:], yts[:, 0:128], wp_s[:, 0:512], True, False)
                    mm(po[:], yts[:, 128:256], wp_s[:, 512:1024], False, True)
                    os_t = osp.tile([128, 512], bf16, tag="os", name=f"os{qt}")
                    nc.vector.tensor_copy(os_t[:], po[:])
                    nc.sync.dma_start(out[qt * 128 : (qt + 1) * 128, :], os_t[:])

                # chores: (deadline qt, pe_ns, thunk) consumed between att
                # tiles so the PE always has independent work while exp()
                # runs; pop ~a tile-period's worth of PE work each time.
                chores = []

                def pop_chore(qt):
                    if chores and chores[0][0] <= qt + 1:
                        chores.pop(0)[2]()

                def flush_chores(qt):
                    while chores and chores[0][0] <= qt:
                        chores.pop(0)[2]()

                def emit_attention(qt):
                    tiles = qt_tiles(qt)
                    yacc = yaccp.tile([128, 260], f32, tag="yacc", name=f"yacc{qt}")
                    ptiles = []
                    for t, (tb, used) in enumerate(tiles):
                        at = attp.tile([128, BPT * 128], f32, tag="att", name=f"at{qt}_{t}")
                        emit_scores(qt, at, tb)
                        if t == 0:
                            # zero-init the accumulator bank with a 1-row
                            # matmul, deferred past the first scores so it
                            # never head-blocks the PE at qtile start
                            mm(yacc[:, 0:260], zer_s[:], zbias_s[:, 0:260],
                               True, False, skip_group_check=True)
                        pt = ptp.tile([128, BPT * 128], bf16, tag="p", name=f"p{qt}_{t}")
                        nc.scalar.activation(pt[:, 0 : used * 128], at[:, 0 : used * 128], EXP)
                        ptiles.append((pt, tb))
                        pop_chore(qt)
                        if t >= 1:
                            emit_pv_tile(qt, yacc, *ptiles[t - 1])
                    emit_pv_tile(qt, yacc, *ptiles[-1])
                    emit_fin(qt, yacc)
                    pop_chore(qt)

                zbias_s = cp.tile([1, 260], bf16)
                nc.vector.memset(zbias_s[:], 0.0)

                # ---- prelude: chains needed by the first query tiles ----
                # park early chains in the idle att-psum banks to run them
                # in parallel; the rest flow through the sm pool.
                pre1 = attp.tile([128, BPT * 128], f32, tag="att", name="pre1")
                emit_q_chain(pre1[:, 0:512], 0, 0, nc.vector.tensor_copy, 0, 128)
                pre2 = attp.tile([128, BPT * 128], f32, tag="att", name="pre2")
                emit_q_chain(pre2[:, 0:512], 0, 1, nc.vector.tensor_copy, 0, 128)
                emit_k_chain(pre1[:, 512:1024], 0, 0, nc.scalar.copy)
                emit_k_chain(pre2[:, 512:1024], 0, 1, nc.scalar.copy)
                emit_unit(("v", 0))

                # remaining chains with deadlines (emit before attention(d))
                chores.append((1, 860, lambda: emit_unit(("q0r", 0))))
                chores.append((1, 860, lambda: emit_unit(("q0r", 1))))
                for si in range(1, len(kspans)):
                    lo = kspans[si][0] // 128
                    d = next((q for q in range(NQT) if nt_c[q] > lo), NQT - 1)
                    chores.append((d, 860, lambda s=si: emit_unit(("k", s, 0))))
                    chores.append((d, 860, lambda s=si: emit_unit(("k", s, 1))))
                for kb in range(1, nvt):
                    d = next((q for q in range(NQT) if nt_c[q] > kb), NQT - 1)
                    chores.append((d, 860, lambda k=kb: emit_unit(("v", k))))
                for qc in range(1, 4):
                    chores.append((4 * qc, 860, lambda q=qc: emit_unit(("q", q, 0))))
                    chores.append((4 * qc, 860, lambda q=qc: emit_unit(("q", q, 1))))
                chores.sort(key=lambda x: x[0])

                for qt in range(NQT):
                    flush_chores(qt)
                    emit_attention(qt)
                    if qt >= 1:
                        # y-transpose and projection of the previous qtile
                        # ride the chore queue to fill later PE gaps
                        chores.append((qt + (1 if qt < 12 else 2), 150, lambda q=qt - 1: emit_trans(q)))
                        chores.append((min(qt + 12, NQT - 1) if qt < 12 else qt + 12, 430, lambda q=qt - 1: emit_proj(q)))
                        chores.sort(key=lambda x: x[0])
                flush_chores(10**9)
                emit_trans(NQT - 1)
                emit_proj(NQT - 1)
                assert not chores and not ysb_t and not yts_t, (
                    "unemitted work left behind",
                    len(chores), list(ysb_t), list(yts_t),
                )
    _split_multi_waits(nc, mybir)
    return nc


def _host_inputs(x, mask, Wq, bq, Wk, bk, Wv, bv, Wp, bp, sched):
    """Build the per-core input maps."""
    import ml_dtypes

    bf = ml_dtypes.bfloat16
    f8 = ml_dtypes.float8_e4m3
    it_c, nt_c, nvt, pairs = sched
    naux = len(pairs)
    KW = nvt * 128
    NW = 512 + naux * 128
    scale = 1.0 / math.sqrt(D)

    # query-side penalty pattern aq4 [34, 512]: -PEN where row > lf(q),
    # replicated over the 4 head slots; row 32 = always-mask, row 33 = pad.
    lf = np.arange(128) // NOBJ
    r = np.arange(34)[:, None]
    aq1 = np.where((r > lf[None, :]) & (r < 33), np.float32(-PEN), np.float32(0.0))
    aq4 = np.tile(aq1, (1, 4))  # [34, 512]

    ident = np.eye(128, dtype=np.float32)
    ones_host = np.ones((1, 4 * T), np.float32)

    in_maps = []
    for c in range(NCORES):
        b, g = divmod(c, 2)
        ch = slice(g * 256, (g + 1) * 256)
        vpos = np.nonzero(mask[b])[0]
        nv = len(vpos)
        kf = vpos // NOBJ
        # compacted x for K/V, zero-padded
        xc_h = np.zeros((C, KW), np.float32)
        xc_h[:, :nv] = x[b].T[:, vpos]
        # aux: [34, NW] -> DoubleRow layout [17, 2, NW]
        aux_h = np.zeros((34, NW), np.float32)
        aux_h[:, 0:512] = aq4
        for p, (qt, v) in enumerate(pairs):
            f0 = 32 * qt
            col0 = 512 + p * 128
            for kk in range(128):
                j = v * 128 + kk
                if j >= nv:
                    aux_h[32, col0 + kk] = 1.0  # padding: always masked
                    continue
                i = int(kf[j]) - f0
                if i <= 0:
                    continue  # interior key: no penalty
                aux_h[min(i, 32), col0 + kk] = 1.0
        aux_dr = aux_h.reshape(2, 17, NW).transpose(1, 0, 2).reshape(17, 2 * NW)

        wq_h = np.ascontiguousarray((Wq[ch, :] * scale).T)  # [C, 256]
        wk_h = np.ascontiguousarray(Wk[ch, :].T)
        wv_h = np.ascontiguousarray(Wv[ch, :].T)
        wp_h = np.ascontiguousarray(Wp[:, ch].T)  # [256, 512]
        # pack [128, 4096]: per-kc 256-wide blocks for wq/wk/wv, rc-major wp
        wb_h = np.zeros((128, 4096), np.float32)
        for kc in range(4):
            wb_h[:, kc * 256 : (kc + 1) * 256] = wq_h[kc * 128 : (kc + 1) * 128, :]
            wb_h[:, 1024 + kc * 256 : 1024 + (kc + 1) * 256] = wk_h[
                kc * 128 : (kc + 1) * 128, :
            ]
            wb_h[:, 2048 + kc * 256 : 2048 + (kc + 1) * 256] = wv_h[
                kc * 128 : (kc + 1) * 128, :
            ]
        for rc in range(2):
            wb_h[:, 3072 + rc * 512 : 3072 + (rc + 1) * 512] = wp_h[
                rc * 128 : (rc + 1) * 128, :
            ]

        # kx[h] = xc^T (Wk_h^T (s bq_h)) : folds the Q bias into a 65th K row
        bqs = bq[ch] * scale
        kx_h = np.zeros((4, KW), np.float32)
        for h in range(HPC):
            wkh = Wk[ch, :][h * 64 : (h + 1) * 64, :]  # [64, C]
            w_vec = wkh.T @ bqs[h * 64 : (h + 1) * 64]  # [C]
            kx_h[h] = xc_h.T @ w_vec

        in_maps.append(
            {
                "xt": np.ascontiguousarray(x[b].T).astype(bf),
                "xc": xc_h.astype(bf),
                "wb": wb_h.astype(bf),
                "onesd": ones_host.astype(bf),
                "kxd": kx_h.astype(bf),
                "identd": ident.astype(bf),
                "auxd": aux_dr.astype(f8),
            }
        )
    return in_maps


def kernel(x, mask, Wq, bq, Wk, bk, Wv, bv, Wp, bp):
    from concourse.bass_utils import run_bass_kernel_spmd

    mask = np.asarray(mask)
    sched = _schedule(mask)
    key = ("nc", sched)
    if key not in _CACHE:
        _CACHE[key] = _build_program(*sched)
        _CACHE["nc"] = _CACHE[key]  # for test harness introspection
    nc = _CACHE[key]

    x, Wq, bq, Wk, bk = map(np.asarray, (x, Wq, bq, Wk, bk))
    Wv, bv, Wp, bp = map(np.asarray, (Wv, bv, Wp, bp))
    in_maps = _host_inputs(x, mask, Wq, bq, Wk, bk, Wv, bv, Wp, bp, sched)
    res = run_bass_kernel_spmd(nc, in_maps, core_ids=list(range(NCORES)))
    outs = [res.results[c]["out"] for c in range(NCORES)]
    const = bp + bv @ Wp.T  # bias contributions folded out of the device pass
    y = np.empty((B, T, C), np.float32)
    for b in range(B):
        y[b] = (
            outs[2 * b].astype(np.float32)
            + outs[2 * b + 1].astype(np.float32)
            + const[None, :]
        )
    return y


# revision 102
# speedup vs baseline: 1.0056x; 1.0008x over previous
"""Trainium2 Bass kernel v4: compacted-key block-causal masked attention.

Module: y = proj(softmax(mask(QK^T/sqrt(D))) V) for B=4, T=2048, C=512, H=8,
with a frame-block-causal mask (frame = t//4) and a per-key validity mask.

Sharding: 8 cores = 4 batches x 2 head-groups (4 heads each); host sums the
two partial output projections per batch and adds the projection bias.

v4 restructures v3 around the TimelineSim cost model:
- K bias is dropped entirely (it only shifts each softmax row uniformly) and
  the Q bias is folded into a host-computed 65th K row (kx = xc^T Wk^T bq s)
  against a constant-ones 65th Q row, so Q/K projection chains need plain
  PSUM->SBUF copies instead of bias adds; scores contract over 65 rows,
  which is free on the PE (cost scales with the free dim only).
- The frame-causality penalty for boundary tiles is a single fp8 DoubleRow
  matmul per (qtile, keytile) pair ([17,2]-contraction one-hot against a
  [17,2,512] query-side pattern) - exact, since all values are 0/1/-192.
- V bias and validity-mask multiplies are gone: padding keys are forced to
  exp()=0 by the penalty rows, and bv's contribution is the constant
  bv @ Wp^T added on the host.
- y^T for the output projection comes from PE transpose-matmuls against an
  identity tile instead of serialized DMA transposes; softmax scaling is one
  DVE multiply with a stride-0 broadcast of the reciprocal denominators.
- Output is written bf16 and summed on the host in f32.
- Emission interleaves projection chains, transposes and output projections
  ("chores", output projections deferred up to 9 qtiles) between att tiles
  so the PE always has independent work while an exp() is in flight; input
  DMAs are merged/ordered against the serialized HWDGE (ones/kx rows ride
  the Pool engine's SWDGE path instead).
"""

import math

import numpy as np

B, T, C = 4, 2048, 512
H, NOBJ, D = 8, 4, 64
NCORES = 8
HPC = 4  # heads per core
NQT = 16  # query tiles of 128
BPT = 8  # score blocks per PSUM att tile ([128, 1024] = 2 banks)
PEN = 192.0  # mask penalty (exactly representable in fp8 e4m3)

_CACHE = {}


def _apply_tile_patch(tile_mod, mybir):
    """walrus in this container rejects >1 semaphore wait per instruction;
    spread the TileContext tail-drain waits over sync NOPs (the rest of the
    module is handled by _split_multi_waits after lowering)."""
    import bass_rust

    if getattr(tile_mod.TileContext, "_drain_patched", False):
        return

    def _drain_and_barrier(self, tick_clock, wait_clock):
        nc = self.nc
        drain_inst = nc.sync.drain()
        wait_clock.add_sem_waits(
            drain_inst.ins, bass_rust.ScopedClock({None: tick_clock.global_clock})
        )
        waits = list(drain_inst.ins.sync_info.on_wait or [])
        if len(waits) > 1:
            drain_inst.ins.sync_info.on_wait = []
            for w in waits:
                nop = nc.sync.nop(nofuse=True)
                nop.ins.sync_info = mybir.SyncInfo(on_wait=[w], on_update=[])
            nc.sync.drain()
        nc.all_engine_barrier()
        assert self.sems is not None
        popped = nc._tile_sem_poison_stack.pop()
        assert popped is self._sem_poison
        nc.clear_and_free_semaphores(list(self.sems.allocated().values()))
        nc.all_engine_barrier()

    tile_mod.TileContext._drain_and_barrier = _drain_and_barrier
    tile_mod.TileContext._drain_patched = True


def _split_multi_waits(nc, mybir):
    """Post-pass: for every instruction carrying more than one semaphore
    wait, hoist the extra waits onto same-engine NOPs inserted immediately
    before it (engines execute serially, so blocking at the NOP is
    equivalent)."""
    nonce = 0
    for fn in nc.m.functions:
        for blk in fn.blocks:
            insts = list(blk.instructions)
            out = []
            changed = False
            for ins in insts:
                si = ins.sync_info
                waits = list(si.on_wait) if si and si.on_wait else []
                if len(waits) > 1:
                    changed = True
                    for w in waits[:-1]:
                        nop = mybir.InstNoOp(
                            name=f"I-waitsplit-{nonce}", ins=[], outs=[]
                        )
                        nonce += 1
                        nop.engine = ins.engine
                        nop.sync_info = mybir.SyncInfo(on_wait=[w], on_update=[])
                        nc.register_instruction(nop, overwrite=True)
                        out.append(nop)
                    ins.sync_info.on_wait = waits[-1:]
                out.append(ins)
            if changed:
                blk.instructions = out


def _schedule(mask):
    """Common (max-over-batch) per-qtile interior/total tile counts."""
    it_c, nt_c = [], []
    kfs = []
    for b in range(B):
        vpos = np.nonzero(mask[b])[0]
        kfs.append(vpos // NOBJ)
    for qt in range(NQT):
        f0, flast = 32 * qt, 32 * qt + 31
        its, nts = [], []
        for b in range(B):
            nf = int(np.searchsorted(kfs[b], f0, "right"))
            na = int(np.searchsorted(kfs[b], flast, "right"))
            its.append(nf // 128)
            nts.append(-(-na // 128))
        it_c.append(min(its))
        nt_c.append(max(max(nts), 1))
    nvt = nt_c[-1]
    pairs = []  # (qt, v) -> aux column index order
    for qt in range(NQT):
        for v in range(it_c[qt], nt_c[qt]):
            pairs.append((qt, v))
    return tuple(it_c), tuple(nt_c), nvt, tuple(pairs)


def _kspans(KW):
    """K projection span widths; a short first span unblocks qt0 early."""
    spans = [(0, 128)]
    off = 128
    while off < KW:
        w = min(512, KW - off)
        spans.append((off, w))
        off += w
    return spans


def _build_program(it_c, nt_c, nvt, pairs):
    import concourse.bass as bass
    import concourse.mybir as mybir
    import concourse.tile as tile

    _apply_tile_patch(tile, mybir)

    f32 = mybir.dt.float32
    bf16 = mybir.dt.bfloat16
    fp8 = mybir.dt.float8e4
    EXP = mybir.ActivationFunctionType.Exp
    DR = mybir.MatmulPerfMode.DoubleRow
    naux = len(pairs)
    pair_idx = {p: i for i, p in enumerate(pairs)}
    KW = nvt * 128  # compacted key width
    NW = 512 + naux * 128  # aux free width per half

    nc = bass.Bass(trn_type="TRN2")

    xt = nc.dram_tensor("xt", [C, T], bf16, kind="ExternalInput")
    xc = nc.dram_tensor("xc", [C, KW], bf16, kind="ExternalInput")
    wb = nc.dram_tensor("wb", [128, 4096], bf16, kind="ExternalInput")
    onesd = nc.dram_tensor("onesd", [1, 4 * T], bf16, kind="ExternalInput")
    kxd = nc.dram_tensor("kxd", [4, KW], bf16, kind="ExternalInput")
    identd = nc.dram_tensor("identd", [128, 128], bf16, kind="ExternalInput")
    auxd = nc.dram_tensor("auxd", [17, 2 * NW], fp8, kind="ExternalInput")
    out = nc.dram_tensor("out", [T, C], bf16, kind="ExternalOutput")

    def mm(o, lhsT, rhs, start, stop, **kw):
        nc.tensor.matmul(o, lhsT, rhs, start=start, stop=stop, **kw)

    kspans = _kspans(KW)

    with nc.allow_low_precision(
        reason="bf16/fp8 matmul inputs; PSUM accumulation stays fp32"
    ), tile.TileContext(nc) as tc:
        with tc.tile_pool(name="const", bufs=1) as cp:
            w_s = cp.tile([128, 4096], bf16)
            xts = cp.tile([128, 4, T], bf16)
            xcs = cp.tile([128, 4, KW], bf16)
            ident = cp.tile([128, 128], bf16)
            aux_s = cp.tile([17, 2, NW], fp8)
            zer_s = cp.tile([1, 128], bf16)
            qtd = cp.tile([65, 4 * T], bf16)
            ktd = cp.tile([65, 4 * KW], bf16)
            v4 = cp.tile([128, nvt, 4, 65], bf16)

            wq_s = w_s[:, 0:1024]
            wk_s = w_s[:, 1024:2048]
            wv_s = w_s[:, 2048:3072]
            wp_s = w_s[:, 3072:4096]

            # ---- input DMAs, ordered by first use (HWDGE serializes) ----
            xtv = xt[:].rearrange("(k p) t -> p k t", p=128)
            xcv = xc[:].rearrange("(k p) t -> p k t", p=128)
            nc.gpsimd.dma_start(ktd[64:65, :], kxd[:])
            nc.gpsimd.dma_start(qtd[64:65, :], onesd[:])
            nc.sync.dma_start(w_s[:, 0:1024], wb[:, 0:1024])
            nc.sync.dma_start(xts[:, :, 0:128], xtv[:, :, 0:128])
            nc.sync.dma_start(xcs[:, :, 0:128], xcv[:, :, 0:128])
            nc.sync.dma_start(w_s[:, 1024:2048], wb[:, 1024:2048])
            nc.sync.dma_start(w_s[:, 2048:3072], wb[:, 2048:3072])
            nc.sync.dma_start(aux_s[:, :, :], auxd[:])
            nc.sync.dma_start(xts[:, :, 128:512], xtv[:, :, 128:512])
            nc.sync.dma_start(xcs[:, :, 128:640], xcv[:, :, 128:640])
            nc.sync.dma_start(xcs[:, :, 640:KW], xcv[:, :, 640:KW])
            nc.sync.dma_start(ident[:], identd[:])
            nc.sync.dma_start(w_s[:, 3072:4096], wb[:, 3072:4096])
            nc.sync.dma_start(xts[:, :, 512:1280], xtv[:, :, 512:1280])
            nc.sync.dma_start(xts[:, :, 1280:2048], xtv[:, :, 1280:2048])
            nc.vector.memset(zer_s[:], 0.0)
            # denominator columns of v4 are constant 1
            nc.vector.memset(v4[:, :, :, 64:65], 1.0)

            with tc.tile_pool(name="attp", bufs=2, space="PSUM") as attp, tc.tile_pool(
                name="yaccp", bufs=2, space="PSUM"
            ) as yaccp, tc.tile_pool(
                name="smp", bufs=2, space="PSUM"
            ) as smp, tc.tile_pool(name="ptp", bufs=3) as ptp, tc.tile_pool(
                name="ysbp", bufs=4
            ) as ysbp, tc.tile_pool(name="rrp", bufs=4) as rrp, tc.tile_pool(
                name="ytsp", bufs=12
            ) as ytsp, tc.tile_pool(name="osp", bufs=12) as osp:
                nonce = [0]

                def emit_q_chain(ps, qc, rc, copy_eng, c0=0, c1=512):
                    w = c1 - c0
                    for kc in range(4):
                        mm(
                            ps[:, 0:w],
                            wq_s[:, kc * 256 + rc * 128 : kc * 256 + rc * 128 + 128],
                            xts[:, kc, qc * 512 + c0 : qc * 512 + c1],
                            kc == 0,
                            kc == 3,
                        )
                    for hh in range(2):
                        h = 2 * rc + hh
                        copy_eng(
                            qtd[0:64, h * T + qc * 512 + c0 : h * T + qc * 512 + c1],
                            ps[hh * 64 : (hh + 1) * 64, 0:w],
                        )

                def emit_k_chain(ps, si, rc, copy_eng):
                    off, w = kspans[si]
                    for kc in range(4):
                        mm(
                            ps[:, 0:w],
                            wk_s[:, kc * 256 + rc * 128 : kc * 256 + rc * 128 + 128],
                            xcs[:, kc, off : off + w],
                            kc == 0,
                            kc == 3,
                        )
                    for hh in range(2):
                        h = 2 * rc + hh
                        copy_eng(
                            ktd[0:64, h * KW + off : h * KW + off + w],
                            ps[hh * 64 : (hh + 1) * 64, 0:w],
                        )

                def emit_v_chain(ps, kb, copy_eng):
                    for kc in range(4):
                        mm(
                            ps[:, 0:256],
                            xcs[:, kc, kb * 128 : kb * 128 + 128],
                            wv_s[:, kc * 256 : (kc + 1) * 256],
                            kc == 0,
                            kc == 3,
                        )
                    copy_eng(v4[:, kb, :, 0:64], ps[:, 0:256])

                def emit_unit(u, ps=None):
                    if ps is None:
                        ps = smp.tile([128, 512], f32, tag="sm", name=f"u{nonce[0]}")
                        nonce[0] += 1
                    kind = u[0]
                    if kind == "q":
                        emit_q_chain(ps, u[1], u[2], nc.vector.tensor_copy)
                    elif kind == "q0r":
                        emit_q_chain(ps, 0, u[1], nc.vector.tensor_copy, 128, 512)
                    elif kind == "k":
                        emit_k_chain(ps, u[1], u[2], nc.vector.tensor_copy)
                    else:
                        emit_v_chain(ps, u[1], nc.vector.tensor_copy)

                def qt_tiles(qt):
                    """Slot items per att tile: interior head-blocks first,
                    then 4-aligned boundary groups."""
                    items = [("blk", h, kb) for h in range(HPC) for kb in range(it_c[qt])]
                    for v in range(it_c[qt], nt_c[qt]):
                        items.append(("grp", v))
                    tiles = []
                    cur, used = [], 0
                    for itm in items:
                        sz = 4 if itm[0] == "grp" else 1
                        if used + sz > BPT:
                            tiles.append((cur, used))
                            cur, used = [], 0
                        cur.append((used, itm))
                        used += sz
                    if cur:
                        tiles.append((cur, used))
                    return tiles

                def emit_scores(qt, at, take):
                    for slot, itm in take:
                        if itm[0] == "blk":
                            _, h, kb = itm
                            mm(
                                at[:, slot * 128 : (slot + 1) * 128],
                                ktd[0:65, h * KW + kb * 128 : h * KW + (kb + 1) * 128],
                                qtd[0:65, h * T + qt * 128 : h * T + (qt + 1) * 128],
                                True,
                                True,
                            )
                        else:
                            v = itm[1]
                            for hh in range(HPC):
                                mm(
                                    at[:, (slot + hh) * 128 : (slot + hh + 1) * 128],
                                    ktd[0:65, hh * KW + v * 128 : hh * KW + (v + 1) * 128],
                                    qtd[0:65, hh * T + qt * 128 : hh * T + (qt + 1) * 128],
                                    hh == 0,
                                    False,
                                    skip_group_check=True,
                                )
                            p = pair_idx[(qt, v)]
                            mm(
                                at[:, slot * 128 : (slot + 4) * 128],
                                aux_s[:, :, 512 + p * 128 : 512 + (p + 1) * 128],
                                aux_s[:, :, 0:512],
                                False,
                                True,
                                perf_mode=DR,
                                skip_group_check=True,
                            )

                def emit_pv_tile(qt, yacc, pt, take):
                    nt = nt_c[qt]
                    for slot, itm in take:
                        if itm[0] == "blk":
                            blocks = [(slot, itm[1], itm[2])]
                        else:
                            blocks = [(slot + hh, hh, itm[1]) for hh in range(HPC)]
                        for sl, h, kb in blocks:
                            mm(
                                yacc[:, h * 65 : h * 65 + 65],
                                pt[:, sl * 128 : (sl + 1) * 128],
                                v4[:, kb, h, :],
                                False,
                                kb == nt - 1,
                                skip_group_check=True,
                            )

                ysb_t = {}
                yts_t = {}

                def emit_fin(qt, yacc):
                    """Recip the denominators, scale all heads into bf16 with
                    one broadcast multiply."""
                    rr = rrp.tile([128, 4], f32, tag="rr", name=f"rr{qt}")
                    nc.vector.reciprocal(rr[:], yacc[:, 64:260:65])
                    ysb = ysbp.tile([128, 256], bf16, tag="ysb", name=f"ysb{qt}")
                    yv = yacc.rearrange("p (h d) -> p h d", h=4)[:, :, 0:64]
                    nc.vector.tensor_mul(
                        ysb[:].rearrange("p (h d) -> p h d", h=4),
                        yv,
                        rr[:].unsqueeze(2).broadcast_to([128, 4, 64]),
                    )
                    ysb_t[qt] = ysb

                def emit_trans(qt):
                    ysb = ysb_t.pop(qt)
                    ytp = yaccp.tile([128, 256], bf16, tag="yacc", name=f"ytp{qt}")
                    nc.tensor.transpose(ytp[:, 0:128], ysb[:, 0:128], ident[:])
                    nc.tensor.transpose(ytp[:, 128:256], ysb[:, 128:256], ident[:])
                    yts = ytsp.tile([128, 256], bf16, tag="yts", name=f"yts{qt}")
                    nc.vector.tensor_copy(yts[:], ytp[:])
                    yts_t[qt] = yts

                def emit_proj(qt):
                    yts = yts_t.pop(qt)
                    po = smp.tile([128, 512], f32, tag="sm", name=f"po{qt}")
                    mm(po[:], yts[:, 0:128], wp_s[:, 0:512], True, False)
                    mm(po[:], yts[:, 128:256], wp_s[:, 512:1024], False, True)
                    os_t = osp.tile([128, 512], bf16, tag="os", name=f"os{qt}")
                    nc.vector.tensor_copy(os_t[:], po[:])
                    nc.sync.dma_start(out[qt * 128 : (qt + 1) * 128, :], os_t[:])

                # chores: (deadline qt, pe_ns, thunk) consumed between att
                # tiles so the PE always has independent work while exp()
                # runs; pop ~a tile-period's worth of PE work each time.
                chores = []

                def pop_chore(qt):
                    if chores and chores[0][0] <= qt + 1:
                        chores.pop(0)[2]()

                def flush_chores(qt):
                    while chores and chores[0][0] <= qt:
                        chores.pop(0)[2]()

                def emit_attention(qt):
                    tiles = qt_tiles(qt)
                    yacc = yaccp.tile([128, 260], f32, tag="yacc", name=f"yacc{qt}")
                    ptiles = []
                    for t, (tb, used) in enumerate(tiles):
                        at = attp.tile([128, BPT * 128], f32, tag="att", name=f"at{qt}_{t}")
                        emit_scores(qt, at, tb)
                        if t == 0:
                            # zero-init the accumulator bank with a 1-row
                            # matmul, deferred past the first scores so it
                            # never head-blocks the PE at qtile start
                            mm(yacc[:, 0:260], zer_s[:], zbias_s[:, 0:260],
                               True, False, skip_group_check=True)
                        pt = ptp.tile([128, BPT * 128], bf16, tag="p", name=f"p{qt}_{t}")
                        nc.scalar.activation(pt[:, 0 : used * 128], at[:, 0 : used * 128], EXP)
                        ptiles.append((pt, tb))
                        pop_chore(qt)
                        if t >= 1:
                            emit_pv_tile(qt, yacc, *ptiles[t - 1])
                    emit_pv_tile(qt, yacc, *ptiles[-1])
                    emit_fin(qt, yacc)
                    pop_chore(qt)

                zbias_s = cp.tile([1, 260], bf16)
                nc.vector.memset(zbias_s[:], 0.0)

                # ---- prelude: chains needed by the first query tiles ----
                # park early chains in the idle att-psum banks to run them
                # in parallel; the rest flow through the sm pool.
                pre1 = attp.tile([128, BPT * 128], f32, tag="att", name="pre1")
                emit_q_chain(pre1[:, 0:512], 0, 0, nc.vector.tensor_copy, 0, 128)
                pre2 = attp.tile([128, BPT * 128], f32, tag="att", name="pre2")
                emit_q_chain(pre2[:, 0:512], 0, 1, nc.vector.tensor_copy, 0, 128)
                emit_k_chain(pre1[:, 512:1024], 0, 0, nc.scalar.copy)
                emit_k_chain(pre2[:, 512:1024], 0, 1, nc.scalar.copy)
                emit_unit(("v", 0))

                # remaining chains with deadlines (emit before attention(d))
                chores.append((1, 860, lambda: emit_unit(("q0r", 0))))
                chores.append((1, 860, lambda: emit_unit(("q0r", 1))))
                for si in range(1, len(kspans)):
                    lo = kspans[si][0] // 128
                    d = next((q for q in range(NQT) if nt_c[q] > lo), NQT - 1)
                    chores.append((d, 860, lambda s=si: emit_unit(("k", s, 0))))
                    chores.append((d, 860, lambda s=si: emit_unit(("k", s, 1))))
                for kb in range(1, nvt):
                    d = next((q for q in range(NQT) if nt_c[q] > kb), NQT - 1)
                    chores.append((d, 860, lambda k=kb: emit_unit(("v", k))))
                for qc in range(1, 4):
                    chores.append((4 * qc, 860, lambda q=qc: emit_unit(("q", q, 0))))
                    chores.append((4 * qc, 860, lambda q=qc: emit_unit(("q", q, 1))))
                chores.sort(key=lambda x: x[0])

                for qt in range(NQT):
                    flush_chores(qt)
                    emit_attention(qt)
                    if qt >= 1:
                        # y-transpose and projection of the previous qtile
                        # ride the chore queue to fill later PE gaps
                        chores.append((qt + (1 if qt < 12 else 2), 150, lambda q=qt - 1: emit_trans(q)))
                        chores.append((min(qt + 11, NQT - 1) if qt < 12 else qt + 11, 430, lambda q=qt - 1: emit_proj(q)))
                        chores.sort(key=lambda x: x[0])
                flush_chores(10**9)
                emit_trans(NQT - 1)
                emit_proj(NQT - 1)
                assert not chores and not ysb_t and not yts_t, (
                    "unemitted work left behind",
                    len(chores), list(ysb_t), list(yts_t),
                )
    _split_multi_waits(nc, mybir)
    return nc


def _host_inputs(x, mask, Wq, bq, Wk, bk, Wv, bv, Wp, bp, sched):
    """Build the per-core input maps."""
    import ml_dtypes

    bf = ml_dtypes.bfloat16
    f8 = ml_dtypes.float8_e4m3
    it_c, nt_c, nvt, pairs = sched
    naux = len(pairs)
    KW = nvt * 128
    NW = 512 + naux * 128
    scale = 1.0 / math.sqrt(D)

    # query-side penalty pattern aq4 [34, 512]: -PEN where row > lf(q),
    # replicated over the 4 head slots; row 32 = always-mask, row 33 = pad.
    lf = np.arange(128) // NOBJ
    r = np.arange(34)[:, None]
    aq1 = np.where((r > lf[None, :]) & (r < 33), np.float32(-PEN), np.float32(0.0))
    aq4 = np.tile(aq1, (1, 4))  # [34, 512]

    ident = np.eye(128, dtype=np.float32)
    ones_host = np.ones((1, 4 * T), np.float32)

    in_maps = []
    for c in range(NCORES):
        b, g = divmod(c, 2)
        ch = slice(g * 256, (g + 1) * 256)
        vpos = np.nonzero(mask[b])[0]
        nv = len(vpos)
        kf = vpos // NOBJ
        # compacted x for K/V, zero-padded
        xc_h = np.zeros((C, KW), np.float32)
        xc_h[:, :nv] = x[b].T[:, vpos]
        # aux: [34, NW] -> DoubleRow layout [17, 2, NW]
        aux_h = np.zeros((34, NW), np.float32)
        aux_h[:, 0:512] = aq4
        for p, (qt, v) in enumerate(pairs):
            f0 = 32 * qt
            col0 = 512 + p * 128
            for kk in range(128):
                j = v * 128 + kk
                if j >= nv:
                    aux_h[32, col0 + kk] = 1.0  # padding: always masked
                    continue
                i = int(kf[j]) - f0
                if i <= 0:
                    continue  # interior key: no penalty
                aux_h[min(i, 32), col0 + kk] = 1.0
        aux_dr = aux_h.reshape(2, 17, NW).transpose(1, 0, 2).reshape(17, 2 * NW)

        wq_h = np.ascontiguousarray((Wq[ch, :] * scale).T)  # [C, 256]
        wk_h = np.ascontiguousarray(Wk[ch, :].T)
        wv_h = np.ascontiguousarray(Wv[ch, :].T)
        wp_h = np.ascontiguousarray(Wp[:, ch].T)  # [256, 512]
        # pack [128, 4096]: per-kc 256-wide blocks for wq/wk/wv, rc-major wp
        wb_h = np.zeros((128, 4096), np.float32)
        for kc in range(4):
            wb_h[:, kc * 256 : (kc + 1) * 256] = wq_h[kc * 128 : (kc + 1) * 128, :]
            wb_h[:, 1024 + kc * 256 : 1024 + (kc + 1) * 256] = wk_h[
                kc * 128 : (kc + 1) * 128, :
            ]
            wb_h[:, 2048 + kc * 256 : 2048 + (kc + 1) * 256] = wv_h[
                kc * 128 : (kc + 1) * 128, :
            ]
        for rc in range(2):
            wb_h[:, 3072 + rc * 512 : 3072 + (rc + 1) * 512] = wp_h[
                rc * 128 : (rc + 1) * 128, :
            ]

        # kx[h] = xc^T (Wk_h^T (s bq_h)) : folds the Q bias into a 65th K row
        bqs = bq[ch] * scale
        kx_h = np.zeros((4, KW), np.float32)
        for h in range(HPC):
            wkh = Wk[ch, :][h * 64 : (h + 1) * 64, :]  # [64, C]
            w_vec = wkh.T @ bqs[h * 64 : (h + 1) * 64]  # [C]
            kx_h[h] = xc_h.T @ w_vec

        in_maps.append(
            {
                "xt": np.ascontiguousarray(x[b].T).astype(bf),
                "xc": xc_h.astype(bf),
                "wb": wb_h.astype(bf),
                "onesd": ones_host.astype(bf),
                "kxd": kx_h.astype(bf),
                "identd": ident.astype(bf),
                "auxd": aux_dr.astype(f8),
            }
        )
    return in_maps


def kernel(x, mask, Wq, bq, Wk, bk, Wv, bv, Wp, bp):
    from concourse.bass_utils import run_bass_kernel_spmd

    mask = np.asarray(mask)
    sched = _schedule(mask)
    key = ("nc", sched)
    if key not in _CACHE:
        _CACHE[key] = _build_program(*sched)
        _CACHE["nc"] = _CACHE[key]  # for test harness introspection
    nc = _CACHE[key]

    x, Wq, bq, Wk, bk = map(np.asarray, (x, Wq, bq, Wk, bk))
    Wv, bv, Wp, bp = map(np.asarray, (Wv, bv, Wp, bp))
    in_maps = _host_inputs(x, mask, Wq, bq, Wk, bk, Wv, bv, Wp, bp, sched)
    res = run_bass_kernel_spmd(nc, in_maps, core_ids=list(range(NCORES)))
    outs = [res.results[c]["out"] for c in range(NCORES)]
    const = bp + bv @ Wp.T  # bias contributions folded out of the device pass
    y = np.empty((B, T, C), np.float32)
    for b in range(B):
        y[b] = (
            outs[2 * b].astype(np.float32)
            + outs[2 * b + 1].astype(np.float32)
            + const[None, :]
        )
    return y


# revision 108
# speedup vs baseline: 1.0098x; 1.0041x over previous
"""Trainium2 Bass kernel v4: compacted-key block-causal masked attention.

Module: y = proj(softmax(mask(QK^T/sqrt(D))) V) for B=4, T=2048, C=512, H=8,
with a frame-block-causal mask (frame = t//4) and a per-key validity mask.

Sharding: 8 cores = 4 batches x 2 head-groups (4 heads each); host sums the
two partial output projections per batch and adds the projection bias.

v4 restructures v3 around the TimelineSim cost model:
- K bias is dropped entirely (it only shifts each softmax row uniformly) and
  the Q bias is folded into a host-computed 65th K row (kx = xc^T Wk^T bq s)
  against a constant-ones 65th Q row, so Q/K projection chains need plain
  PSUM->SBUF copies instead of bias adds; scores contract over 65 rows,
  which is free on the PE (cost scales with the free dim only).
- The frame-causality penalty for boundary tiles is a single fp8 DoubleRow
  matmul per (qtile, keytile) pair ([17,2]-contraction one-hot against a
  [17,2,512] query-side pattern) - exact, since all values are 0/1/-192.
- V bias and validity-mask multiplies are gone: padding keys are forced to
  exp()=0 by the penalty rows, and bv's contribution is the constant
  bv @ Wp^T added on the host.
- y^T for the output projection comes from PE transpose-matmuls against an
  identity tile instead of serialized DMA transposes; softmax scaling is one
  DVE multiply with a stride-0 broadcast of the reciprocal denominators.
- Output is written bf16 and summed on the host in f32.
- Emission interleaves projection chains, transposes and output projections
  ("chores", output projections deferred up to 9 qtiles) between att tiles
  so the PE always has independent work while an exp() is in flight; input
  DMAs are merged/ordered against the serialized HWDGE (ones/kx rows ride
  the Pool engine's SWDGE path instead).
"""

import math

import numpy as np

B, T, C = 4, 2048, 512
H, NOBJ, D = 8, 4, 64
NCORES = 8
HPC = 4  # heads per core
NQT = 16  # query tiles of 128
BPT = 8  # score blocks per PSUM att tile ([128, 1024] = 2 banks)
PEN = 192.0  # mask penalty (exactly representable in fp8 e4m3)

_CACHE = {}


def _apply_tile_patch(tile_mod, mybir):
    """walrus in this container rejects >1 semaphore wait per instruction;
    spread the TileContext tail-drain waits over sync NOPs (the rest of the
    module is handled by _split_multi_waits after lowering)."""
    import bass_rust

    if getattr(tile_mod.TileContext, "_drain_patched", False):
        return

    def _drain_and_barrier(self, tick_clock, wait_clock):
        nc = self.nc
        drain_inst = nc.sync.drain()
        wait_clock.add_sem_waits(
            drain_inst.ins, bass_rust.ScopedClock({None: tick_clock.global_clock})
        )
        waits = list(drain_inst.ins.sync_info.on_wait or [])
        if len(waits) > 1:
            drain_inst.ins.sync_info.on_wait = []
            for w in waits:
                nop = nc.sync.nop(nofuse=True)
                nop.ins.sync_info = mybir.SyncInfo(on_wait=[w], on_update=[])
            nc.sync.drain()
        nc.all_engine_barrier()
        assert self.sems is not None
        popped = nc._tile_sem_poison_stack.pop()
        assert popped is self._sem_poison
        nc.clear_and_free_semaphores(list(self.sems.allocated().values()))
        nc.all_engine_barrier()

    tile_mod.TileContext._drain_and_barrier = _drain_and_barrier
    tile_mod.TileContext._drain_patched = True


def _split_multi_waits(nc, mybir):
    """Post-pass: for every instruction carrying more than one semaphore
    wait, hoist the extra waits onto same-engine NOPs inserted immediately
    before it (engines execute serially, so blocking at the NOP is
    equivalent)."""
    nonce = 0
    for fn in nc.m.functions:
        for blk in fn.blocks:
            insts = list(blk.instructions)
            out = []
            changed = False
            for ins in insts:
                si = ins.sync_info
                waits = list(si.on_wait) if si and si.on_wait else []
                if len(waits) > 1:
                    changed = True
                    for w in waits[:-1]:
                        nop = mybir.InstNoOp(
                            name=f"I-waitsplit-{nonce}", ins=[], outs=[]
                        )
                        nonce += 1
                        nop.engine = ins.engine
                        nop.sync_info = mybir.SyncInfo(on_wait=[w], on_update=[])
                        nc.register_instruction(nop, overwrite=True)
                        out.append(nop)
                    ins.sync_info.on_wait = waits[-1:]
                out.append(ins)
            if changed:
                blk.instructions = out


def _schedule(mask):
    """Common (max-over-batch) per-qtile interior/total tile counts."""
    it_c, nt_c = [], []
    kfs = []
    for b in range(B):
        vpos = np.nonzero(mask[b])[0]
        kfs.append(vpos // NOBJ)
    for qt in range(NQT):
        f0, flast = 32 * qt, 32 * qt + 31
        its, nts = [], []
        for b in range(B):
            nf = int(np.searchsorted(kfs[b], f0, "right"))
            na = int(np.searchsorted(kfs[b], flast, "right"))
            its.append(nf // 128)
            nts.append(-(-na // 128))
        it_c.append(min(its))
        nt_c.append(max(max(nts), 1))
    nvt = nt_c[-1]
    pairs = []  # (qt, v) -> aux column index order
    for qt in range(NQT):
        for v in range(it_c[qt], nt_c[qt]):
            pairs.append((qt, v))
    return tuple(it_c), tuple(nt_c), nvt, tuple(pairs)


def _kspans(KW):
    """K projection span widths; a short first span unblocks qt0 early."""
    spans = [(0, 128)]
    off = 128
    while off < KW:
        w = min(512, KW - off)
        spans.append((off, w))
        off += w
    return spans


def _build_program(it_c, nt_c, nvt, pairs):
    import concourse.bass as bass
    import concourse.mybir as mybir
    import concourse.tile as tile

    _apply_tile_patch(tile, mybir)

    f32 = mybir.dt.float32
    bf16 = mybir.dt.bfloat16
    fp8 = mybir.dt.float8e4
    EXP = mybir.ActivationFunctionType.Exp
    DR = mybir.MatmulPerfMode.DoubleRow
    naux = len(pairs)
    pair_idx = {p: i for i, p in enumerate(pairs)}
    KW = nvt * 128  # compacted key width
    NW = 512 + naux * 128  # aux free width per half

    nc = bass.Bass(trn_type="TRN2")

    xt = nc.dram_tensor("xt", [C, T], bf16, kind="ExternalInput")
    xc = nc.dram_tensor("xc", [C, KW], bf16, kind="ExternalInput")
    wb = nc.dram_tensor("wb", [128, 4096], bf16, kind="ExternalInput")
    onesd = nc.dram_tensor("onesd", [1, 4 * T], bf16, kind="ExternalInput")
    kxd = nc.dram_tensor("kxd", [4, KW], bf16, kind="ExternalInput")
    identd = nc.dram_tensor("identd", [128, 128], bf16, kind="ExternalInput")
    auxd = nc.dram_tensor("auxd", [17, 2 * NW], fp8, kind="ExternalInput")
    out = nc.dram_tensor("out", [T, C], bf16, kind="ExternalOutput")

    def mm(o, lhsT, rhs, start, stop, **kw):
        nc.tensor.matmul(o, lhsT, rhs, start=start, stop=stop, **kw)

    kspans = _kspans(KW)

    with nc.allow_low_precision(
        reason="bf16/fp8 matmul inputs; PSUM accumulation stays fp32"
    ), tile.TileContext(nc) as tc:
        with tc.tile_pool(name="const", bufs=1) as cp:
            w_s = cp.tile([128, 4096], bf16)
            xts = cp.tile([128, 4, T], bf16)
            xcs = cp.tile([128, 4, KW], bf16)
            ident = cp.tile([128, 128], bf16)
            aux_s = cp.tile([17, 2, NW], fp8)
            zer_s = cp.tile([1, 128], bf16)
            qtd = cp.tile([65, 4 * T], bf16)
            ktd = cp.tile([65, 4 * KW], bf16)
            v4 = cp.tile([128, nvt, 4, 65], bf16)

            wq_s = w_s[:, 0:1024]
            wk_s = w_s[:, 1024:2048]
            wv_s = w_s[:, 2048:3072]
            wp_s = w_s[:, 3072:4096]

            # ---- input DMAs, ordered by first use (HWDGE serializes) ----
            xtv = xt[:].rearrange("(k p) t -> p k t", p=128)
            xcv = xc[:].rearrange("(k p) t -> p k t", p=128)
            nc.gpsimd.dma_start(ktd[64:65, :], kxd[:])
            nc.gpsimd.dma_start(qtd[64:65, :], onesd[:])
            nc.sync.dma_start(w_s[:, 0:1024], wb[:, 0:1024])
            nc.sync.dma_start(xts[:, :, 0:128], xtv[:, :, 0:128])
            nc.sync.dma_start(xcs[:, :, 0:128], xcv[:, :, 0:128])
            nc.sync.dma_start(w_s[:, 1024:2048], wb[:, 1024:2048])
            nc.sync.dma_start(w_s[:, 2048:3072], wb[:, 2048:3072])
            nc.sync.dma_start(aux_s[:, :, :], auxd[:])
            nc.sync.dma_start(xts[:, :, 128:512], xtv[:, :, 128:512])
            nc.sync.dma_start(xcs[:, :, 128:640], xcv[:, :, 128:640])
            nc.sync.dma_start(xcs[:, :, 640:KW], xcv[:, :, 640:KW])
            nc.sync.dma_start(ident[:], identd[:])
            nc.sync.dma_start(w_s[:, 3072:4096], wb[:, 3072:4096])
            nc.sync.dma_start(xts[:, :, 512:1280], xtv[:, :, 512:1280])
            nc.sync.dma_start(xts[:, :, 1280:2048], xtv[:, :, 1280:2048])
            nc.vector.memset(zer_s[:], 0.0)
            # denominator columns of v4 are constant 1
            nc.vector.memset(v4[:, :, :, 64:65], 1.0)

            with tc.tile_pool(name="attp", bufs=2, space="PSUM") as attp, tc.tile_pool(
                name="yaccp", bufs=2, space="PSUM"
            ) as yaccp, tc.tile_pool(
                name="smp", bufs=2, space="PSUM"
            ) as smp, tc.tile_pool(name="ptp", bufs=3) as ptp, tc.tile_pool(
                name="ysbp", bufs=4
            ) as ysbp, tc.tile_pool(name="rrp", bufs=4) as rrp, tc.tile_pool(
                name="ytsp", bufs=12
            ) as ytsp, tc.tile_pool(name="osp", bufs=12) as osp:
                nonce = [0]

                def emit_q_chain(ps, qc, rc, copy_eng, c0=0, c1=512):
                    w = c1 - c0
                    for kc in range(4):
                        mm(
                            ps[:, 0:w],
                            wq_s[:, kc * 256 + rc * 128 : kc * 256 + rc * 128 + 128],
                            xts[:, kc, qc * 512 + c0 : qc * 512 + c1],
                            kc == 0,
                            kc == 3,
                        )
                    for hh in range(2):
                        h = 2 * rc + hh
                        copy_eng(
                            qtd[0:64, h * T + qc * 512 + c0 : h * T + qc * 512 + c1],
                            ps[hh * 64 : (hh + 1) * 64, 0:w],
                        )

                def emit_k_chain(ps, si, rc, copy_eng):
                    off, w = kspans[si]
                    for kc in range(4):
                        mm(
                            ps[:, 0:w],
                            wk_s[:, kc * 256 + rc * 128 : kc * 256 + rc * 128 + 128],
                            xcs[:, kc, off : off + w],
                            kc == 0,
                            kc == 3,
                        )
                    for hh in range(2):
                        h = 2 * rc + hh
                        copy_eng(
                            ktd[0:64, h * KW + off : h * KW + off + w],
                            ps[hh * 64 : (hh + 1) * 64, 0:w],
                        )

                def emit_v_chain(ps, kb, copy_eng):
                    for kc in range(4):
                        mm(
                            ps[:, 0:256],
                            xcs[:, kc, kb * 128 : kb * 128 + 128],
                            wv_s[:, kc * 256 : (kc + 1) * 256],
                            kc == 0,
                            kc == 3,
                        )
                    copy_eng(v4[:, kb, :, 0:64], ps[:, 0:256])

                def emit_unit(u, ps=None):
                    if ps is None:
                        ps = smp.tile([128, 512], f32, tag="sm", name=f"u{nonce[0]}")
                        nonce[0] += 1
                    kind = u[0]
                    if kind == "q":
                        emit_q_chain(ps, u[1], u[2], nc.vector.tensor_copy)
                    elif kind == "q0r":
                        emit_q_chain(ps, 0, u[1], nc.vector.tensor_copy, 128, 512)
                    elif kind == "k":
                        emit_k_chain(ps, u[1], u[2], nc.vector.tensor_copy)
                    else:
                        emit_v_chain(ps, u[1], nc.vector.tensor_copy)

                def qt_tiles(qt):
                    """Slot items per att tile: interior head-blocks first,
                    then 4-aligned boundary groups."""
                    items = [("blk", h, kb) for h in range(HPC) for kb in range(it_c[qt])]
                    for v in range(it_c[qt], nt_c[qt]):
                        items.append(("grp", v))
                    tiles = []
                    cur, used = [], 0
                    for itm in items:
                        sz = 4 if itm[0] == "grp" else 1
                        if used + sz > BPT:
                            tiles.append((cur, used))
                            cur, used = [], 0
                        cur.append((used, itm))
                        used += sz
                    if cur:
                        tiles.append((cur, used))
                    return tiles

                def emit_scores(qt, at, take):
                    for slot, itm in take:
                        if itm[0] == "blk":
                            _, h, kb = itm
                            mm(
                                at[:, slot * 128 : (slot + 1) * 128],
                                ktd[0:65, h * KW + kb * 128 : h * KW + (kb + 1) * 128],
                                qtd[0:65, h * T + qt * 128 : h * T + (qt + 1) * 128],
                                True,
                                True,
                            )
                        else:
                            v = itm[1]
                            for hh in range(HPC):
                                mm(
                                    at[:, (slot + hh) * 128 : (slot + hh + 1) * 128],
                                    ktd[0:65, hh * KW + v * 128 : hh * KW + (v + 1) * 128],
                                    qtd[0:65, hh * T + qt * 128 : hh * T + (qt + 1) * 128],
                                    hh == 0,
                                    False,
                                    skip_group_check=True,
                                )
                            p = pair_idx[(qt, v)]
                            mm(
                                at[:, slot * 128 : (slot + 4) * 128],
                                aux_s[:, :, 512 + p * 128 : 512 + (p + 1) * 128],
                                aux_s[:, :, 0:512],
                                False,
                                True,
                                perf_mode=DR,
                                skip_group_check=True,
                            )

                def emit_pv_tile(qt, yacc, pt, take):
                    nt = nt_c[qt]
                    for slot, itm in take:
                        if itm[0] == "blk":
                            blocks = [(slot, itm[1], itm[2])]
                        else:
                            blocks = [(slot + hh, hh, itm[1]) for hh in range(HPC)]
                        for sl, h, kb in blocks:
                            mm(
                                yacc[:, h * 65 : h * 65 + 65],
                                pt[:, sl * 128 : (sl + 1) * 128],
                                v4[:, kb, h, :],
                                False,
                                kb == nt - 1,
                                skip_group_check=True,
                            )

                ysb_t = {}
                yts_t = {}

                def emit_fin(qt, yacc):
                    """Recip the denominators, scale all heads into bf16 with
                    one broadcast multiply."""
                    rr = rrp.tile([128, 4], f32, tag="rr", name=f"rr{qt}")
                    nc.vector.reciprocal(rr[:], yacc[:, 64:260:65])
                    ysb = ysbp.tile([128, 256], bf16, tag="ysb", name=f"ysb{qt}")
                    yv = yacc.rearrange("p (h d) -> p h d", h=4)[:, :, 0:64]
                    nc.vector.tensor_mul(
                        ysb[:].rearrange("p (h d) -> p h d", h=4),
                        yv,
                        rr[:].unsqueeze(2).broadcast_to([128, 4, 64]),
                    )
                    ysb_t[qt] = ysb

                def emit_trans(qt):
                    ysb = ysb_t.pop(qt)
                    ytp = yaccp.tile([128, 256], bf16, tag="yacc", name=f"ytp{qt}")
                    nc.tensor.transpose(ytp[:, 0:128], ysb[:, 0:128], ident[:])
                    nc.tensor.transpose(ytp[:, 128:256], ysb[:, 128:256], ident[:])
                    yts = ytsp.tile([128, 256], bf16, tag="yts", name=f"yts{qt}")
                    nc.vector.tensor_copy(yts[:], ytp[:])
                    yts_t[qt] = yts

                def emit_proj(qt):
                    yts = yts_t.pop(qt)
                    po = smp.tile([128, 512], f32, tag="sm", name=f"po{qt}")
                    mm(po[:], yts[:, 0:128], wp_s[:, 0:512], True, False)
                    mm(po[:], yts[:, 128:256], wp_s[:, 512:1024], False, True)
                    os_t = osp.tile([128, 512], bf16, tag="os", name=f"os{qt}")
                    nc.vector.tensor_copy(os_t[:], po[:])
                    nc.sync.dma_start(out[qt * 128 : (qt + 1) * 128, :], os_t[:])

                # chores: (deadline qt, pe_ns, thunk) consumed between att
                # tiles so the PE always has independent work while exp()
                # runs; pop ~a tile-period's worth of PE work each time.
                chores = []

                def pop_chore(qt):
                    if chores and chores[0][0] <= qt + 1:
                        chores.pop(0)[2]()

                def flush_chores(qt):
                    while chores and chores[0][0] <= qt:
                        chores.pop(0)[2]()

                def emit_attention(qt):
                    tiles = qt_tiles(qt)
                    yacc = yaccp.tile([128, 260], f32, tag="yacc", name=f"yacc{qt}")
                    ptiles = []
                    for t, (tb, used) in enumerate(tiles):
                        at = attp.tile([128, BPT * 128], f32, tag="att", name=f"at{qt}_{t}")
                        emit_scores(qt, at, tb)
                        if t == 0:
                            # zero-init the accumulator bank with a 1-row
                            # matmul, deferred past the first scores so it
                            # never head-blocks the PE at qtile start
                            mm(yacc[:, 0:260], zer_s[:], zbias_s[:, 0:260],
                               True, False, skip_group_check=True)
                        pt = ptp.tile([128, BPT * 128], bf16, tag="p", name=f"p{qt}_{t}")
                        nc.scalar.activation(pt[:, 0 : used * 128], at[:, 0 : used * 128], EXP)
                        ptiles.append((pt, tb))
                        pop_chore(qt)
                        if t >= 1:
                            emit_pv_tile(qt, yacc, *ptiles[t - 1])
                    emit_pv_tile(qt, yacc, *ptiles[-1])
                    emit_fin(qt, yacc)
                    pop_chore(qt)

                zbias_s = cp.tile([1, 260], bf16)
                nc.vector.memset(zbias_s[:], 0.0)

                # ---- prelude: chains needed by the first query tiles ----
                # park early chains in the idle att-psum banks to run them
                # in parallel; the rest flow through the sm pool.
                pre1 = attp.tile([128, BPT * 128], f32, tag="att", name="pre1")
                emit_q_chain(pre1[:, 0:512], 0, 0, nc.vector.tensor_copy, 0, 128)
                pre2 = attp.tile([128, BPT * 128], f32, tag="att", name="pre2")
                emit_q_chain(pre2[:, 0:512], 0, 1, nc.vector.tensor_copy, 0, 128)
                emit_k_chain(pre1[:, 512:1024], 0, 0, nc.scalar.copy)
                emit_k_chain(pre2[:, 512:1024], 0, 1, nc.scalar.copy)
                emit_unit(("v", 0))

                # remaining chains with deadlines (emit before attention(d))
                chores.append((1, 860, lambda: emit_unit(("q0r", 0))))
                chores.append((1, 860, lambda: emit_unit(("q0r", 1))))
                for si in range(1, len(kspans)):
                    lo = kspans[si][0] // 128
                    d = next((q for q in range(NQT) if nt_c[q] > lo), NQT - 1)
                    chores.append((d, 860, lambda s=si: emit_unit(("k", s, 0))))
                    chores.append((d, 860, lambda s=si: emit_unit(("k", s, 1))))
                for kb in range(1, nvt):
                    d = next((q for q in range(NQT) if nt_c[q] > kb), NQT - 1)
                    chores.append((d, 860, lambda k=kb: emit_unit(("v", k))))
                for qc in range(1, 4):
                    d = 4 * qc - (1 if qc <= 2 else 0)
                    chores.append((d, 860, lambda q=qc: emit_unit(("q", q, 0))))
                    chores.append((d, 860, lambda q=qc: emit_unit(("q", q, 1))))
                chores.sort(key=lambda x: x[0])

                for qt in range(NQT):
                    flush_chores(qt)
                    emit_attention(qt)
                    if qt >= 1:
                        # y-transpose and projection of the previous qtile
                        # ride the chore queue to fill later PE gaps
                        chores.append((qt + (1 if qt < 12 else 2), 150, lambda q=qt - 1: emit_trans(q)))
                        chores.append((min(qt + 11, NQT - 1) if qt < 12 else qt + 11, 430, lambda q=qt - 1: emit_proj(q)))
                        chores.sort(key=lambda x: x[0])
                flush_chores(10**9)
                emit_trans(NQT - 1)
                emit_proj(NQT - 1)
                assert not chores and not ysb_t and not yts_t, (
                    "unemitted work left behind",
                    len(chores), list(ysb_t), list(yts_t),
                )
    _split_multi_waits(nc, mybir)
    return nc


def _host_inputs(x, mask, Wq, bq, Wk, bk, Wv, bv, Wp, bp, sched):
    """Build the per-core input maps."""
    import ml_dtypes

    bf = ml_dtypes.bfloat16
    f8 = ml_dtypes.float8_e4m3
    it_c, nt_c, nvt, pairs = sched
    naux = len(pairs)
    KW = nvt * 128
    NW = 512 + naux * 128
    scale = 1.0 / math.sqrt(D)

    # query-side penalty pattern aq4 [34, 512]: -PEN where row > lf(q),
    # replicated over the 4 head slots; row 32 = always-mask, row 33 = pad.
    lf = np.arange(128) // NOBJ
    r = np.arange(34)[:, None]
    aq1 = np.where((r > lf[None, :]) & (r < 33), np.float32(-PEN), np.float32(0.0))
    aq4 = np.tile(aq1, (1, 4))  # [34, 512]

    ident = np.eye(128, dtype=np.float32)
    ones_host = np.ones((1, 4 * T), np.float32)

    in_maps = []
    for c in range(NCORES):
        b, g = divmod(c, 2)
        ch = slice(g * 256, (g + 1) * 256)
        vpos = np.nonzero(mask[b])[0]
        nv = len(vpos)
        kf = vpos // NOBJ
        # compacted x for K/V, zero-padded
        xc_h = np.zeros((C, KW), np.float32)
        xc_h[:, :nv] = x[b].T[:, vpos]
        # aux: [34, NW] -> DoubleRow layout [17, 2, NW]
        aux_h = np.zeros((34, NW), np.float32)
        aux_h[:, 0:512] = aq4
        for p, (qt, v) in enumerate(pairs):
            f0 = 32 * qt
            col0 = 512 + p * 128
            for kk in range(128):
                j = v * 128 + kk
                if j >= nv:
                    aux_h[32, col0 + kk] = 1.0  # padding: always masked
                    continue
                i = int(kf[j]) - f0
                if i <= 0:
                    continue  # interior key: no penalty
                aux_h[min(i, 32), col0 + kk] = 1.0
        aux_dr = aux_h.reshape(2, 17, NW).transpose(1, 0, 2).reshape(17, 2 * NW)

        wq_h = np.ascontiguousarray((Wq[ch, :] * scale).T)  # [C, 256]
        wk_h = np.ascontiguousarray(Wk[ch, :].T)
        wv_h = np.ascontiguousarray(Wv[ch, :].T)
        wp_h = np.ascontiguousarray(Wp[:, ch].T)  # [256, 512]
        # pack [128, 4096]: per-kc 256-wide blocks for wq/wk/wv, rc-major wp
        wb_h = np.zeros((128, 4096), np.float32)
        for kc in range(4):
            wb_h[:, kc * 256 : (kc + 1) * 256] = wq_h[kc * 128 : (kc + 1) * 128, :]
            wb_h[:, 1024 + kc * 256 : 1024 + (kc + 1) * 256] = wk_h[
                kc * 128 : (kc + 1) * 128, :
            ]
            wb_h[:, 2048 + kc * 256 : 2048 + (kc + 1) * 256] = wv_h[
                kc * 128 : (kc + 1) * 128, :
            ]
        for rc in range(2):
            wb_h[:, 3072 + rc * 512 : 3072 + (rc + 1) * 512] = wp_h[
                rc * 128 : (rc + 1) * 128, :
            ]

        # kx[h] = xc^T (Wk_h^T (s bq_h)) : folds the Q bias into a 65th K row
        bqs = bq[ch] * scale
        kx_h = np.zeros((4, KW), np.float32)
        for h in range(HPC):
            wkh = Wk[ch, :][h * 64 : (h + 1) * 64, :]  # [64, C]
            w_vec = wkh.T @ bqs[h * 64 : (h + 1) * 64]  # [C]
            kx_h[h] = xc_h.T @ w_vec

        in_maps.append(
            {
                "xt": np.ascontiguousarray(x[b].T).astype(bf),
                "xc": xc_h.astype(bf),
                "wb": wb_h.astype(bf),
                "onesd": ones_host.astype(bf),
                "kxd": kx_h.astype(bf),
                "identd": ident.astype(bf),
                "auxd": aux_dr.astype(f8),
            }
        )
    return in_maps


def kernel(x, mask, Wq, bq, Wk, bk, Wv, bv, Wp, bp):
    from concourse.bass_utils import run_bass_kernel_spmd

    mask = np.asarray(mask)
    sched = _schedule(mask)
    key = ("nc", sched)
    if key not in _CACHE:
        _CACHE[key] = _build_program(*sched)
        _CACHE["nc"] = _CACHE[key]  # for test harness introspection
    nc = _CACHE[key]

    x, Wq, bq, Wk, bk = map(np.asarray, (x, Wq, bq, Wk, bk))
    Wv, bv, Wp, bp = map(np.asarray, (Wv, bv, Wp, bp))
    in_maps = _host_inputs(x, mask, Wq, bq, Wk, bk, Wv, bv, Wp, bp, sched)
    res = run_bass_kernel_spmd(nc, in_maps, core_ids=list(range(NCORES)))
    outs = [res.results[c]["out"] for c in range(NCORES)]
    const = bp + bv @ Wp.T  # bias contributions folded out of the device pass
    y = np.empty((B, T, C), np.float32)
    for b in range(B):
        y[b] = (
            outs[2 * b].astype(np.float32)
            + outs[2 * b + 1].astype(np.float32)
            + const[None, :]
        )
    return y


# revision 114
# speedup vs baseline: 1.0140x; 1.0042x over previous
"""Trainium2 Bass kernel v4: compacted-key block-causal masked attention.

Module: y = proj(softmax(mask(QK^T/sqrt(D))) V) for B=4, T=2048, C=512, H=8,
with a frame-block-causal mask (frame = t//4) and a per-key validity mask.

Sharding: 8 cores = 4 batches x 2 head-groups (4 heads each); host sums the
two partial output projections per batch and adds the projection bias.

v4 restructures v3 around the TimelineSim cost model:
- K bias is dropped entirely (it only shifts each softmax row uniformly) and
  the Q bias is folded into a host-computed 65th K row (kx = xc^T Wk^T bq s)
  against a constant-ones 65th Q row, so Q/K projection chains need plain
  PSUM->SBUF copies instead of bias adds; scores contract over 65 rows,
  which is free on the PE (cost scales with the free dim only).
- The frame-causality penalty for boundary tiles is a single fp8 DoubleRow
  matmul per (qtile, keytile) pair ([17,2]-contraction one-hot against a
  [17,2,512] query-side pattern) - exact, since all values are 0/1/-192.
- V bias and validity-mask multiplies are gone: padding keys are forced to
  exp()=0 by the penalty rows, and bv's contribution is the constant
  bv @ Wp^T added on the host.
- y^T for the output projection comes from PE transpose-matmuls against an
  identity tile instead of serialized DMA transposes; softmax scaling is one
  DVE multiply with a stride-0 broadcast of the reciprocal denominators.
- Output is written bf16 and summed on the host in f32.
- Emission interleaves projection chains, transposes and output projections
  ("chores", output projections deferred up to 9 qtiles) between att tiles
  so the PE always has independent work while an exp() is in flight; input
  DMAs are merged/ordered against the serialized HWDGE (ones/kx rows ride
  the Pool engine's SWDGE path instead).
"""

import math

import numpy as np

B, T, C = 4, 2048, 512
H, NOBJ, D = 8, 4, 64
NCORES = 8
HPC = 4  # heads per core
NQT = 16  # query tiles of 128
BPT = 8  # score blocks per PSUM att tile ([128, 1024] = 2 banks)
PEN = 192.0  # mask penalty (exactly representable in fp8 e4m3)

_CACHE = {}


def _apply_tile_patch(tile_mod, mybir):
    """walrus in this container rejects >1 semaphore wait per instruction;
    spread the TileContext tail-drain waits over sync NOPs (the rest of the
    module is handled by _split_multi_waits after lowering)."""
    import bass_rust

    if getattr(tile_mod.TileContext, "_drain_patched", False):
        return

    def _drain_and_barrier(self, tick_clock, wait_clock):
        nc = self.nc
        drain_inst = nc.sync.drain()
        wait_clock.add_sem_waits(
            drain_inst.ins, bass_rust.ScopedClock({None: tick_clock.global_clock})
        )
        waits = list(drain_inst.ins.sync_info.on_wait or [])
        if len(waits) > 1:
            drain_inst.ins.sync_info.on_wait = []
            for w in waits:
                nop = nc.sync.nop(nofuse=True)
                nop.ins.sync_info = mybir.SyncInfo(on_wait=[w], on_update=[])
            nc.sync.drain()
        nc.all_engine_barrier()
        assert self.sems is not None
        popped = nc._tile_sem_poison_stack.pop()
        assert popped is self._sem_poison
        nc.clear_and_free_semaphores(list(self.sems.allocated().values()))
        nc.all_engine_barrier()

    tile_mod.TileContext._drain_and_barrier = _drain_and_barrier
    tile_mod.TileContext._drain_patched = True


def _split_multi_waits(nc, mybir):
    """Post-pass: for every instruction carrying more than one semaphore
    wait, hoist the extra waits onto same-engine NOPs inserted immediately
    before it (engines execute serially, so blocking at the NOP is
    equivalent)."""
    nonce = 0
    for fn in nc.m.functions:
        for blk in fn.blocks:
            insts = list(blk.instructions)
            out = []
            changed = False
            for ins in insts:
                si = ins.sync_info
                waits = list(si.on_wait) if si and si.on_wait else []
                if len(waits) > 1:
                    changed = True
                    for w in waits[:-1]:
                        nop = mybir.InstNoOp(
                            name=f"I-waitsplit-{nonce}", ins=[], outs=[]
                        )
                        nonce += 1
                        nop.engine = ins.engine
                        nop.sync_info = mybir.SyncInfo(on_wait=[w], on_update=[])
                        nc.register_instruction(nop, overwrite=True)
                        out.append(nop)
                    ins.sync_info.on_wait = waits[-1:]
                out.append(ins)
            if changed:
                blk.instructions = out


def _schedule(mask):
    """Common (max-over-batch) per-qtile interior/total tile counts."""
    it_c, nt_c = [], []
    kfs = []
    for b in range(B):
        vpos = np.nonzero(mask[b])[0]
        kfs.append(vpos // NOBJ)
    for qt in range(NQT):
        f0, flast = 32 * qt, 32 * qt + 31
        its, nts = [], []
        for b in range(B):
            nf = int(np.searchsorted(kfs[b], f0, "right"))
            na = int(np.searchsorted(kfs[b], flast, "right"))
            its.append(nf // 128)
            nts.append(-(-na // 128))
        it_c.append(min(its))
        nt_c.append(max(max(nts), 1))
    nvt = nt_c[-1]
    pairs = []  # (qt, v) -> aux column index order
    for qt in range(NQT):
        for v in range(it_c[qt], nt_c[qt]):
            pairs.append((qt, v))
    return tuple(it_c), tuple(nt_c), nvt, tuple(pairs)


def _kspans(KW):
    """K projection span widths; a short first span unblocks qt0 early."""
    spans = [(0, 128)]
    off = 128
    while off < KW:
        w = min(512, KW - off)
        spans.append((off, w))
        off += w
    return spans


def _build_program(it_c, nt_c, nvt, pairs):
    import concourse.bass as bass
    import concourse.mybir as mybir
    import concourse.tile as tile

    _apply_tile_patch(tile, mybir)

    f32 = mybir.dt.float32
    bf16 = mybir.dt.bfloat16
    fp8 = mybir.dt.float8e4
    EXP = mybir.ActivationFunctionType.Exp
    DR = mybir.MatmulPerfMode.DoubleRow
    naux = len(pairs)
    pair_idx = {p: i for i, p in enumerate(pairs)}
    KW = nvt * 128  # compacted key width
    NW = 512 + naux * 128  # aux free width per half

    nc = bass.Bass(trn_type="TRN2")

    xt = nc.dram_tensor("xt", [C, T], bf16, kind="ExternalInput")
    xc = nc.dram_tensor("xc", [C, KW], bf16, kind="ExternalInput")
    wb = nc.dram_tensor("wb", [128, 4096], bf16, kind="ExternalInput")
    onesd = nc.dram_tensor("onesd", [1, 4 * T], bf16, kind="ExternalInput")
    kxd = nc.dram_tensor("kxd", [4, KW], bf16, kind="ExternalInput")
    identd = nc.dram_tensor("identd", [128, 128], bf16, kind="ExternalInput")
    auxd = nc.dram_tensor("auxd", [17, 2 * NW], fp8, kind="ExternalInput")
    out = nc.dram_tensor("out", [T, C], bf16, kind="ExternalOutput")

    def mm(o, lhsT, rhs, start, stop, **kw):
        nc.tensor.matmul(o, lhsT, rhs, start=start, stop=stop, **kw)

    kspans = _kspans(KW)

    with nc.allow_low_precision(
        reason="bf16/fp8 matmul inputs; PSUM accumulation stays fp32"
    ), tile.TileContext(nc) as tc:
        with tc.tile_pool(name="const", bufs=1) as cp:
            w_s = cp.tile([128, 4096], bf16)
            xts = cp.tile([128, 4, T], bf16)
            xcs = cp.tile([128, 4, KW], bf16)
            ident = cp.tile([128, 128], bf16)
            aux_s = cp.tile([17, 2, NW], fp8)
            zer_s = cp.tile([1, 128], bf16)
            qtd = cp.tile([65, 4 * T], bf16)
            ktd = cp.tile([65, 4 * KW], bf16)
            v4 = cp.tile([128, nvt, 4, 65], bf16)

            wq_s = w_s[:, 0:1024]
            wk_s = w_s[:, 1024:2048]
            wv_s = w_s[:, 2048:3072]
            wp_s = w_s[:, 3072:4096]

            # ---- input DMAs, ordered by first use (HWDGE serializes) ----
            xtv = xt[:].rearrange("(k p) t -> p k t", p=128)
            xcv = xc[:].rearrange("(k p) t -> p k t", p=128)
            nc.gpsimd.dma_start(ktd[64:65, :], kxd[:])
            nc.gpsimd.dma_start(qtd[64:65, :], onesd[:])
            nc.sync.dma_start(w_s[:, 0:1024], wb[:, 0:1024])
            nc.sync.dma_start(xts[:, :, 0:128], xtv[:, :, 0:128])
            nc.sync.dma_start(xcs[:, :, 0:128], xcv[:, :, 0:128])
            nc.sync.dma_start(w_s[:, 1024:2048], wb[:, 1024:2048])
            nc.sync.dma_start(w_s[:, 2048:3072], wb[:, 2048:3072])
            nc.sync.dma_start(aux_s[:, :, :], auxd[:])
            nc.sync.dma_start(xts[:, :, 128:512], xtv[:, :, 128:512])
            nc.sync.dma_start(xcs[:, :, 128:640], xcv[:, :, 128:640])
            nc.sync.dma_start(xcs[:, :, 640:KW], xcv[:, :, 640:KW])
            nc.sync.dma_start(ident[:], identd[:])
            nc.sync.dma_start(w_s[:, 3072:4096], wb[:, 3072:4096])
            nc.sync.dma_start(xts[:, :, 512:1280], xtv[:, :, 512:1280])
            nc.sync.dma_start(xts[:, :, 1280:2048], xtv[:, :, 1280:2048])
            nc.vector.memset(zer_s[:], 0.0)
            # denominator columns of v4 are constant 1
            nc.vector.memset(v4[:, :, :, 64:65], 1.0)

            with tc.tile_pool(name="attp", bufs=2, space="PSUM") as attp, tc.tile_pool(
                name="yaccp", bufs=2, space="PSUM"
            ) as yaccp, tc.tile_pool(
                name="smp", bufs=2, space="PSUM"
            ) as smp, tc.tile_pool(name="ptp", bufs=3) as ptp, tc.tile_pool(
                name="ysbp", bufs=4
            ) as ysbp, tc.tile_pool(name="rrp", bufs=4) as rrp, tc.tile_pool(
                name="ytsp", bufs=12
            ) as ytsp, tc.tile_pool(name="osp", bufs=12) as osp:
                nonce = [0]

                def emit_q_chain(ps, qc, rc, copy_eng, c0=0, c1=512):
                    w = c1 - c0
                    for kc in range(4):
                        mm(
                            ps[:, 0:w],
                            wq_s[:, kc * 256 + rc * 128 : kc * 256 + rc * 128 + 128],
                            xts[:, kc, qc * 512 + c0 : qc * 512 + c1],
                            kc == 0,
                            kc == 3,
                        )
                    for hh in range(2):
                        h = 2 * rc + hh
                        copy_eng(
                            qtd[0:64, h * T + qc * 512 + c0 : h * T + qc * 512 + c1],
                            ps[hh * 64 : (hh + 1) * 64, 0:w],
                        )

                def emit_k_chain(ps, si, rc, copy_eng):
                    off, w = kspans[si]
                    for kc in range(4):
                        mm(
                            ps[:, 0:w],
                            wk_s[:, kc * 256 + rc * 128 : kc * 256 + rc * 128 + 128],
                            xcs[:, kc, off : off + w],
                            kc == 0,
                            kc == 3,
                        )
                    for hh in range(2):
                        h = 2 * rc + hh
                        copy_eng(
                            ktd[0:64, h * KW + off : h * KW + off + w],
                            ps[hh * 64 : (hh + 1) * 64, 0:w],
                        )

                def emit_v_chain(ps, kb, copy_eng):
                    for kc in range(4):
                        mm(
                            ps[:, 0:256],
                            xcs[:, kc, kb * 128 : kb * 128 + 128],
                            wv_s[:, kc * 256 : (kc + 1) * 256],
                            kc == 0,
                            kc == 3,
                        )
                    copy_eng(v4[:, kb, :, 0:64], ps[:, 0:256])

                def emit_unit(u, ps=None):
                    if ps is None:
                        ps = smp.tile([128, 512], f32, tag="sm", name=f"u{nonce[0]}")
                        nonce[0] += 1
                    kind = u[0]
                    if kind == "q":
                        emit_q_chain(ps, u[1], u[2], nc.vector.tensor_copy)
                    elif kind == "q0r":
                        emit_q_chain(ps, 0, u[1], nc.vector.tensor_copy, 128, 512)
                    elif kind == "k":
                        emit_k_chain(ps, u[1], u[2], nc.vector.tensor_copy)
                    else:
                        emit_v_chain(ps, u[1], nc.vector.tensor_copy)

                def qt_tiles(qt):
                    """Slot items per att tile: interior head-blocks first,
                    then 4-aligned boundary groups."""
                    items = [("blk", h, kb) for h in range(HPC) for kb in range(it_c[qt])]
                    for v in range(it_c[qt], nt_c[qt]):
                        items.append(("grp", v))
                    tiles = []
                    cur, used = [], 0
                    for itm in items:
                        sz = 4 if itm[0] == "grp" else 1
                        if used + sz > BPT:
                            tiles.append((cur, used))
                            cur, used = [], 0
                        cur.append((used, itm))
                        used += sz
                    if cur:
                        tiles.append((cur, used))
                    return tiles

                def emit_scores(qt, at, take):
                    for slot, itm in take:
                        if itm[0] == "blk":
                            _, h, kb = itm
                            mm(
                                at[:, slot * 128 : (slot + 1) * 128],
                                ktd[0:65, h * KW + kb * 128 : h * KW + (kb + 1) * 128],
                                qtd[0:65, h * T + qt * 128 : h * T + (qt + 1) * 128],
                                True,
                                True,
                            )
                        else:
                            v = itm[1]
                            for hh in range(HPC):
                                mm(
                                    at[:, (slot + hh) * 128 : (slot + hh + 1) * 128],
                                    ktd[0:65, hh * KW + v * 128 : hh * KW + (v + 1) * 128],
                                    qtd[0:65, hh * T + qt * 128 : hh * T + (qt + 1) * 128],
                                    hh == 0,
                                    False,
                                    skip_group_check=True,
                                )
                            p = pair_idx[(qt, v)]
                            mm(
                                at[:, slot * 128 : (slot + 4) * 128],
                                aux_s[:, :, 512 + p * 128 : 512 + (p + 1) * 128],
                                aux_s[:, :, 0:512],
                                False,
                                True,
                                perf_mode=DR,
                                skip_group_check=True,
                            )

                def emit_pv_tile(qt, yacc, pt, take):
                    nt = nt_c[qt]
                    for slot, itm in take:
                        if itm[0] == "blk":
                            blocks = [(slot, itm[1], itm[2])]
                        else:
                            blocks = [(slot + hh, hh, itm[1]) for hh in range(HPC)]
                        for sl, h, kb in blocks:
                            mm(
                                yacc[:, h * 65 : h * 65 + 65],
                                pt[:, sl * 128 : (sl + 1) * 128],
                                v4[:, kb, h, :],
                                False,
                                kb == nt - 1,
                                skip_group_check=True,
                            )

                ysb_t = {}
                yts_t = {}

                def emit_fin(qt, yacc):
                    """Recip the denominators, scale all heads into bf16 with
                    one broadcast multiply."""
                    rr = rrp.tile([128, 4], f32, tag="rr", name=f"rr{qt}")
                    nc.vector.reciprocal(rr[:], yacc[:, 64:260:65])
                    ysb = ysbp.tile([128, 256], bf16, tag="ysb", name=f"ysb{qt}")
                    yv = yacc.rearrange("p (h d) -> p h d", h=4)[:, :, 0:64]
                    nc.vector.tensor_mul(
                        ysb[:].rearrange("p (h d) -> p h d", h=4),
                        yv,
                        rr[:].unsqueeze(2).broadcast_to([128, 4, 64]),
                    )
                    ysb_t[qt] = ysb

                def emit_trans(qt):
                    ysb = ysb_t.pop(qt)
                    ytp = yaccp.tile([128, 256], bf16, tag="yacc", name=f"ytp{qt}")
                    nc.tensor.transpose(ytp[:, 0:128], ysb[:, 0:128], ident[:])
                    nc.tensor.transpose(ytp[:, 128:256], ysb[:, 128:256], ident[:])
                    yts = ytsp.tile([128, 256], bf16, tag="yts", name=f"yts{qt}")
                    nc.vector.tensor_copy(yts[:], ytp[:])
                    yts_t[qt] = yts

                def emit_proj(qt):
                    yts = yts_t.pop(qt)
                    po = smp.tile([128, 512], f32, tag="sm", name=f"po{qt}")
                    mm(po[:], yts[:, 0:128], wp_s[:, 0:512], True, False)
                    mm(po[:], yts[:, 128:256], wp_s[:, 512:1024], False, True)
                    os_t = osp.tile([128, 512], bf16, tag="os", name=f"os{qt}")
                    nc.vector.tensor_copy(os_t[:], po[:])
                    nc.sync.dma_start(out[qt * 128 : (qt + 1) * 128, :], os_t[:])

                # chores: (deadline qt, pe_ns, thunk) consumed between att
                # tiles so the PE always has independent work while exp()
                # runs; pop ~a tile-period's worth of PE work each time.
                chores = []

                def pop_chore(qt):
                    if chores and chores[0][0] <= qt + 1:
                        chores.pop(0)[2]()

                def flush_chores(qt):
                    while chores and chores[0][0] <= qt:
                        chores.pop(0)[2]()

                def emit_attention(qt):
                    tiles = qt_tiles(qt)
                    yacc = yaccp.tile([128, 260], f32, tag="yacc", name=f"yacc{qt}")
                    ptiles = []
                    for t, (tb, used) in enumerate(tiles):
                        at = attp.tile([128, BPT * 128], f32, tag="att", name=f"at{qt}_{t}")
                        emit_scores(qt, at, tb)
                        if t == 0:
                            # zero-init the accumulator bank with a 1-row
                            # matmul, deferred past the first scores so it
                            # never head-blocks the PE at qtile start
                            mm(yacc[:, 0:260], zer_s[:], zbias_s[:, 0:260],
                               True, False, skip_group_check=True)
                        pt = ptp.tile([128, BPT * 128], bf16, tag="p", name=f"p{qt}_{t}")
                        nc.scalar.activation(pt[:, 0 : used * 128], at[:, 0 : used * 128], EXP)
                        ptiles.append((pt, tb))
                        pop_chore(qt)
                        if t >= 1:
                            emit_pv_tile(qt, yacc, *ptiles[t - 1])
                    emit_pv_tile(qt, yacc, *ptiles[-1])
                    emit_fin(qt, yacc)
                    pop_chore(qt)

                zbias_s = cp.tile([1, 260], bf16)
                nc.vector.memset(zbias_s[:], 0.0)

                # ---- prelude: chains needed by the first query tiles ----
                # park early chains in the idle att-psum banks to run them
                # in parallel; the rest flow through the sm pool.
                pre1 = attp.tile([128, BPT * 128], f32, tag="att", name="pre1")
                emit_q_chain(pre1[:, 0:512], 0, 0, nc.vector.tensor_copy, 0, 128)
                pre2 = attp.tile([128, BPT * 128], f32, tag="att", name="pre2")
                emit_q_chain(pre2[:, 0:512], 0, 1, nc.vector.tensor_copy, 0, 128)
                emit_k_chain(pre1[:, 512:1024], 0, 0, nc.scalar.copy)
                emit_k_chain(pre2[:, 512:1024], 0, 1, nc.scalar.copy)
                pre3 = attp.tile([128, BPT * 128], f32, tag="att", name="pre3")
                emit_v_chain(pre3[:, 0:512], 0, nc.vector.tensor_copy)

                # remaining chains with deadlines (emit before attention(d))
                chores.append((1, 860, lambda: emit_unit(("q0r", 0))))
                chores.append((1, 860, lambda: emit_unit(("q0r", 1))))
                for si in range(1, len(kspans)):
                    lo = kspans[si][0] // 128
                    d = next((q for q in range(NQT) if nt_c[q] > lo), NQT - 1)
                    chores.append((d, 860, lambda s=si: emit_unit(("k", s, 0))))
                    chores.append((d, 860, lambda s=si: emit_unit(("k", s, 1))))
                for kb in range(1, nvt):
                    d = next((q for q in range(NQT) if nt_c[q] > kb), NQT - 1)
                    chores.append((d, 860, lambda k=kb: emit_unit(("v", k))))
                for qc in range(1, 4):
                    d = 4 * qc - (1 if qc <= 2 else 0)
                    chores.append((d, 860, lambda q=qc: emit_unit(("q", q, 0))))
                    chores.append((d, 860, lambda q=qc: emit_unit(("q", q, 1))))
                chores.sort(key=lambda x: x[0])

                for qt in range(NQT):
                    flush_chores(qt)
                    emit_attention(qt)
                    if qt >= 1:
                        # y-transpose and projection of the previous qtile
                        # ride the chore queue to fill later PE gaps
                        chores.append((qt + (1 if qt < 12 else 2), 150, lambda q=qt - 1: emit_trans(q)))
                        chores.append((min(qt + 11, NQT - 1) if qt < 12 else qt + 11, 430, lambda q=qt - 1: emit_proj(q)))
                        chores.sort(key=lambda x: x[0])
                flush_chores(10**9)
                emit_trans(NQT - 1)
                emit_proj(NQT - 1)
                assert not chores and not ysb_t and not yts_t, (
                    "unemitted work left behind",
                    len(chores), list(ysb_t), list(yts_t),
                )
    _split_multi_waits(nc, mybir)
    return nc


def _host_inputs(x, mask, Wq, bq, Wk, bk, Wv, bv, Wp, bp, sched):
    """Build the per-core input maps."""
    import ml_dtypes

    bf = ml_dtypes.bfloat16
    f8 = ml_dtypes.float8_e4m3
    it_c, nt_c, nvt, pairs = sched
    naux = len(pairs)
    KW = nvt * 128
    NW = 512 + naux * 128
    scale = 1.0 / math.sqrt(D)

    # query-side penalty pattern aq4 [34, 512]: -PEN where row > lf(q),
    # replicated over the 4 head slots; row 32 = always-mask, row 33 = pad.
    lf = np.arange(128) // NOBJ
    r = np.arange(34)[:, None]
    aq1 = np.where((r > lf[None, :]) & (r < 33), np.float32(-PEN), np.float32(0.0))
    aq4 = np.tile(aq1, (1, 4))  # [34, 512]

    ident = np.eye(128, dtype=np.float32)
    ones_host = np.ones((1, 4 * T), np.float32)

    in_maps = []
    for c in range(NCORES):
        b, g = divmod(c, 2)
        ch = slice(g * 256, (g + 1) * 256)
        vpos = np.nonzero(mask[b])[0]
        nv = len(vpos)
        kf = vpos // NOBJ
        # compacted x for K/V, zero-padded
        xc_h = np.zeros((C, KW), np.float32)
        xc_h[:, :nv] = x[b].T[:, vpos]
        # aux: [34, NW] -> DoubleRow layout [17, 2, NW]
        aux_h = np.zeros((34, NW), np.float32)
        aux_h[:, 0:512] = aq4
        for p, (qt, v) in enumerate(pairs):
            f0 = 32 * qt
            col0 = 512 + p * 128
            for kk in range(128):
                j = v * 128 + kk
                if j >= nv:
                    aux_h[32, col0 + kk] = 1.0  # padding: always masked
                    continue
                i = int(kf[j]) - f0
                if i <= 0:
                    continue  # interior key: no penalty
                aux_h[min(i, 32), col0 + kk] = 1.0
        aux_dr = aux_h.reshape(2, 17, NW).transpose(1, 0, 2).reshape(17, 2 * NW)

        wq_h = np.ascontiguousarray((Wq[ch, :] * scale).T)  # [C, 256]
        wk_h = np.ascontiguousarray(Wk[ch, :].T)
        wv_h = np.ascontiguousarray(Wv[ch, :].T)
        wp_h = np.ascontiguousarray(Wp[:, ch].T)  # [256, 512]
        # pack [128, 4096]: per-kc 256-wide blocks for wq/wk/wv, rc-major wp
        wb_h = np.zeros((128, 4096), np.float32)
        for kc in range(4):
            wb_h[:, kc * 256 : (kc + 1) * 256] = wq_h[kc * 128 : (kc + 1) * 128, :]
            wb_h[:, 1024 + kc * 256 : 1024 + (kc + 1) * 256] = wk_h[
                kc * 128 : (kc + 1) * 128, :
            ]
            wb_h[:, 2048 + kc * 256 : 2048 + (kc + 1) * 256] = wv_h[
                kc * 128 : (kc + 1) * 128, :
            ]
        for rc in range(2):
            wb_h[:, 3072 + rc * 512 : 3072 + (rc + 1) * 512] = wp_h[
                rc * 128 : (rc + 1) * 128, :
            ]

        # kx[h] = xc^T (Wk_h^T (s bq_h)) : folds the Q bias into a 65th K row
        bqs = bq[ch] * scale
        kx_h = np.zeros((4, KW), np.float32)
        for h in range(HPC):
            wkh = Wk[ch, :][h * 64 : (h + 1) * 64, :]  # [64, C]
            w_vec = wkh.T @ bqs[h * 64 : (h + 1) * 64]  # [C]
            kx_h[h] = xc_h.T @ w_vec

        in_maps.append(
            {
                "xt": np.ascontiguousarray(x[b].T).astype(bf),
                "xc": xc_h.astype(bf),
                "wb": wb_h.astype(bf),
                "onesd": ones_host.astype(bf),
                "kxd": kx_h.astype(bf),
                "identd": ident.astype(bf),
                "auxd": aux_dr.astype(f8),
            }
        )
    return in_maps


def kernel(x, mask, Wq, bq, Wk, bk, Wv, bv, Wp, bp):
    from concourse.bass_utils import run_bass_kernel_spmd

    mask = np.asarray(mask)
    sched = _schedule(mask)
    key = ("nc", sched)
    if key not in _CACHE:
        _CACHE[key] = _build_program(*sched)
        _CACHE["nc"] = _CACHE[key]  # for test harness introspection
    nc = _CACHE[key]

    x, Wq, bq, Wk, bk = map(np.asarray, (x, Wq, bq, Wk, bk))
    Wv, bv, Wp, bp = map(np.asarray, (Wv, bv, Wp, bp))
    in_maps = _host_inputs(x, mask, Wq, bq, Wk, bk, Wv, bv, Wp, bp, sched)
    res = run_bass_kernel_spmd(nc, in_maps, core_ids=list(range(NCORES)))
    outs = [res.results[c]["out"] for c in range(NCORES)]
    const = bp + bv @ Wp.T  # bias contributions folded out of the device pass
    y = np.empty((B, T, C), np.float32)
    for b in range(B):
        y[b] = (
            outs[2 * b].astype(np.float32)
            + outs[2 * b + 1].astype(np.float32)
            + const[None, :]
        )
    return y
